# revision 1
# baseline (speedup 1.0000x reference)
"""Trainium2 Bass kernel for nn_HRRAdaptedAttention (B=2, S=8192, D=1024).

out = output + gate * irfft(cumsum_s(rfft(k)*rfft(v)) * conj(rfft(q))),
q/k/v = hidden @ W.T + b.

Sharding: (batch, seq) -> 8 chunks of 2048 positions, one per core.
The rfft/irfft are folded into the projection weights on the host
(fk = h @ (Wk.T @ C) etc.), so everything on device is fp32r matmuls,
elementwise complex arithmetic, and a per-frequency cumsum over the
sequence axis (tensor_tensor_scan, [freq->partitions, seq->free] layout).

Launch 1 (per core): h^T -> fk,fv -> kv = fk*fv -> kv chunk to DRAM,
plus per-frequency chunk totals.
Host: 8x[1025] exclusive prefix over chunk totals (causal carry).
Launch 2: scan(kv, initial=prefix) -> mem; fq; Z = mem*conj(fq);
values = Z @ [A;B] (gate folded); res = output + values.
"""

import numpy as np

B, S, D = 2, 8192, 1024
F = 513
NCORES = 8
CHUNK = 2048
PANEL = 512
NPANEL = CHUNK // PANEL
FT = 4                   # 128-row freq tiles f=0..511; f=512 handled apart
NDP = 8
KVROWS = 1152            # kv dram rows: 512 re + 512 im + kvnyq + fqnyq

_cache = {}


def _host_constants(Wq, bq, Wk, bk, Wv, bv, gate):
    d = np.arange(D, dtype=np.float64)
    f = np.arange(F, dtype=np.float64)
    ang = 2.0 * np.pi * np.outer(d, f) / D
    C = np.cos(ang)
    Sm = -np.sin(ang)

    def fold(W, sign_s=1.0):
        Wt = W.T.astype(np.float64)
        return (Wt @ C).astype(np.float32), (sign_s * (Wt @ Sm)).astype(np.float32)

    MkC, MkS = fold(Wk)
    MvC, MvS = fold(Wv)
    MqC, MqS = fold(Wq, sign_s=-1.0)          # conj(fq) folded

    g = float(np.asarray(gate).reshape(-1)[0])
    w = np.full(F, 2.0)
    w[0] = 1.0
    w[512] = 1.0
    scale = (w * g / D)[:, None]
    A = (scale * C.T).astype(np.float32)       # [F, D] coeff for Zre
    Bm = (scale * Sm.T).astype(np.float32)     # [F, D] coeff for Zim

    bk64, bv64, bq64 = (x.astype(np.float64) for x in (bk, bv, bq))
    bias = np.zeros((6, 520), dtype=np.float32)
    bias[0, :F] = (bk64 @ C).astype(np.float32)
    bias[1, :F] = (bk64 @ Sm).astype(np.float32)
    bias[2, :F] = (bv64 @ C).astype(np.float32)
    bias[3, :F] = (bv64 @ Sm).astype(np.float32)
    bias[4, :F] = (bq64 @ C).astype(np.float32)
    bias[5, :F] = (-(bq64 @ Sm)).astype(np.float32)
    return dict(MkC=MkC, MkS=MkS, MvC=MvC, MvS=MvS, MqC=MqC, MqS=MqS,
                A=A, Bm=Bm, bias=bias)


_WAIT_EXEMPT = {
    "InstNoOp", "InstEventSemaphore", "InstUnconditionalBranch",
    "InstRegisterMove", "InstCall", "InstISA",
}


def _legalize_waits(nc, max_waits=1):
    """TRN2 instruction structs hold one sync-wait command; move extra waits
    onto same-engine nops inserted just before the instruction."""
    import bass_rust
    import concourse.mybir as mybir
    ctr = 0
    for fn in nc.m.functions:
        for blk in fn.blocks:
            new = []
            for inst in blk.instructions:
                if (type(inst).__name__ not in _WAIT_EXEMPT
                        and inst.sync_info is not None):
                    waits = list(inst.sync_info.on_wait)
                    if len(waits) > max_waits:
                        for w in waits[:-max_waits]:
                            nop = mybir.InstNoOp(
                                name=f"I-lglnop-{ctr}", ins=[], outs=[])
                            ctr += 1
                            nop.engine = inst.engine
                            nop.sync_info = bass_rust.SyncInfo(
                                on_wait=[w], on_update=[])
                            new.append(nop)
                        inst.sync_info = bass_rust.SyncInfo(
                            on_wait=waits[-max_waits:],
                            on_update=inst.sync_info.on_update)
                new.append(inst)
            blk.instructions = new


def _make_ht(nc, tc, htp, hnp, pst, h_d, identr, p0):
    """Load h[p0:p0+PANEL] and emit h^T tiles [128d, PANEL] (fp32r)."""
    import concourse.mybir as mybir
    F32R = mybir.dt.float32r
    ht = [htp.tile([128, PANEL], F32R, tag=f"ht_{dp}", name=f"ht_{dp}") for dp in range(NDP)]
    for st in range(PANEL // 128):
        hn = hnp.tile([128, D], F32R, tag="hn")
        nc.sync.dma_start(hn[:], h_d.ap()[p0 + st * 128:p0 + (st + 1) * 128, :])
        for dp in range(NDP):
            tp = pst.tile([128, 128], F32R, tag="trps")
            nc.tensor.transpose(tp[:], hn[:, dp * 128:(dp + 1) * 128],
                                identr[:])
            nc.scalar.copy(ht[dp][:, st * 128:(st + 1) * 128], tp[:])
    return ht


def _build_a(has_bias):
    import concourse.bass as bass
    import concourse.mybir as mybir
    import concourse.tile as tile
    F32, F32R = mybir.dt.float32, mybir.dt.float32r
    AT = mybir.AluOpType

    nc = bass.Bass("TRN2", target_bir_lowering=False, debug=False,
                   num_devices=NCORES)
    h_d = nc.dram_tensor("h", [CHUNK, D], F32R, kind="ExternalInput")
    m_d = {nm: nc.dram_tensor(nm, [D, 512], F32R, kind="ExternalInput")
           for nm in ("MkC", "MkS", "MvC", "MvS")}
    mnyq_d = nc.dram_tensor("Mnyq", [D, 2], F32R, kind="ExternalInput")
    identr_d = nc.dram_tensor("identr", [128, 128], F32R, kind="ExternalInput")
    if has_bias:
        bias_d = nc.dram_tensor("biasA", [1, 4 * 520 + 2], F32R,
                                kind="ExternalInput")
        ones_d = nc.dram_tensor("ones", [1, PANEL], F32R, kind="ExternalInput")
    kvd = nc.dram_tensor("kvd", [KVROWS, CHUNK], F32, kind="ExternalOutput")
    tot_d = nc.dram_tensor("totals", [1056, 1], F32, kind="ExternalOutput")
    htd = nc.dram_tensor("htd", [D, CHUNK], F32R, kind="ExternalOutput")

    with tile.TileContext(nc) as tc:
        with (
            tc.tile_pool(name="const", bufs=1) as cp,
            tc.tile_pool(name="wpool", bufs=1) as wp,
            tc.tile_pool(name="ht", bufs=2) as htp,
            tc.tile_pool(name="hnat", bufs=3) as hnp,
            tc.tile_pool(name="work", bufs=3) as wkp,
            tc.tile_pool(name="acc", bufs=1) as accp,
            tc.tile_pool(name="psA", bufs=4, space="PSUM") as psA,
            tc.tile_pool(name="psN", bufs=1, space="PSUM") as psN,
            tc.tile_pool(name="psT", bufs=2, space="PSUM") as pst,
        ):
            identr = cp.tile([128, 128], F32R, tag="identr")
            nc.sync.dma_start(identr[:], identr_d.ap())
            if has_bias:
                bias = cp.tile([1, 4 * 520 + 2], F32R, tag="bias")
                nc.sync.dma_start(bias[:], bias_d.ap())
                ones = cp.tile([1, PANEL], F32R, tag="ones")
                nc.sync.dma_start(ones[:], ones_d.ap())
            mkv = {}
            for nm in ("MkC", "MkS", "MvC", "MvS"):
                for dp in range(NDP):
                    t = wp.tile([128, 512], F32R, tag=f"m_{nm}_{dp}")
                    nc.sync.dma_start(
                        t[:], m_d[nm].ap()[dp * 128:(dp + 1) * 128, :])
                    mkv[(nm, dp)] = t
            mnyq = []
            for dp in range(NDP):
                t = cp.tile([128, 2], F32R, tag=f"mnyq_{dp}")
                nc.sync.dma_start(t[:], mnyq_d.ap()[dp * 128:(dp + 1) * 128, :])
                mnyq.append(t)

            acc = {i: accp.tile([128, 1], F32, tag=f"acc_{i}", name=f"acc_{i}") for i in range(8)}
            accn = accp.tile([1, 1], F32, tag="acc_n")

            for p in range(NPANEL):
                p0 = p * PANEL
                ht = _make_ht(nc, tc, htp, hnp, pst, h_d, identr, p0)
                for dp in range(NDP):
                    nc.sync.dma_start(
                        htd.ap()[dp * 128:(dp + 1) * 128, p0:p0 + PANEL],
                        ht[dp][:])
                for ft in range(FT):
                    ps = {}
                    for i, nm in enumerate(("MkC", "MkS", "MvC", "MvS")):
                        pt = psA.tile([128, PANEL], F32, tag="fwd")
                        for dp in range(NDP):
                            nc.tensor.matmul(
                                pt[:], mkv[(nm, dp)][:, ft * 128:(ft + 1) * 128],
                                ht[dp][:], start=(dp == 0),
                                stop=(dp == NDP - 1 and not has_bias))
                        if has_bias:
                            nc.tensor.matmul(
                                pt[:],
                                bias[:, i * 520 + ft * 128:i * 520 + (ft + 1) * 128],
                                ones[:], start=False, stop=True)
                        ps[nm] = pt
                    fkre = wkp.tile([128, PANEL], F32, tag="fkre")
                    fkim = wkp.tile([128, PANEL], F32, tag="fkim")
                    nc.scalar.copy(fkre[:], ps["MkC"][:])
                    nc.scalar.copy(fkim[:], ps["MkS"][:])
                    t1 = wkp.tile([128, PANEL], F32, tag="t1")
                    t2 = wkp.tile([128, PANEL], F32, tag="t2")
                    kvre = wkp.tile([128, PANEL], F32, tag="kvre")
                    kvim = wkp.tile([128, PANEL], F32, tag="kvim")
                    nc.vector.tensor_tensor(t1[:], fkre[:], ps["MvC"][:],
                                            op=AT.mult)
                    nc.vector.tensor_tensor(t2[:], fkim[:], ps["MvS"][:],
                                            op=AT.mult)
                    nc.vector.tensor_tensor(kvre[:], t1[:], t2[:],
                                            op=AT.subtract)
                    nc.vector.tensor_tensor(t1[:], fkre[:], ps["MvS"][:],
                                            op=AT.mult)
                    nc.vector.tensor_tensor(t2[:], fkim[:], ps["MvC"][:],
                                            op=AT.mult)
                    nc.vector.tensor_tensor(kvim[:], t1[:], t2[:], op=AT.add)
                    nc.sync.dma_start(
                        kvd.ap()[ft * 128:(ft + 1) * 128, p0:p0 + PANEL],
                        kvre[:])
                    nc.sync.dma_start(
                        kvd.ap()[512 + ft * 128:512 + (ft + 1) * 128,
                                 p0:p0 + PANEL], kvim[:])
                    red = wkp.tile([128, 1], F32, tag="red")
                    nc.vector.tensor_reduce(red[:], kvre[:],
                                            axis=mybir.AxisListType.X,
                                            op=AT.add)
                    nc.gpsimd.tensor_tensor(acc[ft][:], acc[ft][:], red[:],
                                            op=AT.add) if p else \
                        nc.gpsimd.tensor_copy(acc[ft][:], red[:])
                    red2 = wkp.tile([128, 1], F32, tag="red")
                    nc.vector.tensor_reduce(red2[:], kvim[:],
                                            axis=mybir.AxisListType.X,
                                            op=AT.add)
                    nc.gpsimd.tensor_tensor(acc[4 + ft][:], acc[4 + ft][:],
                                            red2[:], op=AT.add) if p else \
                        nc.gpsimd.tensor_copy(acc[4 + ft][:], red2[:])
                # nyquist: fk512, fv512 real rows (separate M=1 groups,
                # partition-0 base everywhere)
                pnk = psN.tile([1, PANEL], F32, tag="nyqk")
                pnv = psN.tile([1, PANEL], F32, tag="nyqv")
                for dp in range(NDP):
                    nc.tensor.matmul(pnk[:], mnyq[dp][:, 0:1], ht[dp][:],
                                     start=(dp == 0),
                                     stop=(dp == NDP - 1 and not has_bias))
                for dp in range(NDP):
                    nc.tensor.matmul(pnv[:], mnyq[dp][:, 1:2], ht[dp][:],
                                     start=(dp == 0),
                                     stop=(dp == NDP - 1 and not has_bias))
                if has_bias:
                    nc.tensor.matmul(pnk[:], bias[:, 4 * 520:4 * 520 + 1],
                                     ones[:], start=False, stop=True)
                    nc.tensor.matmul(pnv[:], bias[:, 4 * 520 + 1:4 * 520 + 2],
                                     ones[:], start=False, stop=True)
                nyk = wkp.tile([1, PANEL], F32, tag="nyk")
                nc.scalar.copy(nyk[:], pnk[:])
                kvn = wkp.tile([1, PANEL], F32, tag="kvn")
                nc.vector.tensor_tensor(kvn[:], nyk[:], pnv[:],
                                        op=AT.mult)
                nc.sync.dma_start(kvd.ap()[1024:1025, p0:p0 + PANEL], kvn[:])
                redn = wkp.tile([1, 1], F32, tag="redn")
                nc.vector.tensor_reduce(redn[:], kvn[:],
                                        axis=mybir.AxisListType.X, op=AT.add)
                if p:
                    nc.gpsimd.tensor_tensor(accn[:], accn[:], redn[:],
                                            op=AT.add)
                else:
                    nc.gpsimd.tensor_copy(accn[:], redn[:])

            for i in range(8):
                nc.sync.dma_start(tot_d.ap()[i * 128:(i + 1) * 128, 0:1],
                                  acc[i][:])
            nc.sync.dma_start(tot_d.ap()[1024:1025, 0:1], accn[:])

    _legalize_waits(nc)
    return nc


def _build_b(has_bias):
    import concourse.bass as bass
    import concourse.mybir as mybir
    import concourse.tile as tile
    F32, F32R = mybir.dt.float32, mybir.dt.float32r
    AT = mybir.AluOpType

    nc = bass.Bass("TRN2", target_bir_lowering=False, debug=False,
                   num_devices=NCORES)
    htd = nc.dram_tensor("htd", [D, CHUNK], F32R, kind="ExternalInput")
    kvd = nc.dram_tensor("kvd", [KVROWS, CHUNK], F32, kind="ExternalInput")
    init_d = nc.dram_tensor("init", [1056, 1], F32, kind="ExternalInput")
    outp_d = nc.dram_tensor("outp", [CHUNK, D], F32, kind="ExternalInput")
    m_d = {nm: nc.dram_tensor(nm, [D, 512], F32R, kind="ExternalInput")
           for nm in ("MqC", "MqS")}
    mnyq_d = nc.dram_tensor("Mnyq", [D, 1], F32R, kind="ExternalInput")
    a_d = nc.dram_tensor("A", [512, D], F32R, kind="ExternalInput")
    b_d = nc.dram_tensor("Bm", [512, D], F32R, kind="ExternalInput")
    a512_d = nc.dram_tensor("A512", [1, D], F32R, kind="ExternalInput")
    if has_bias:
        bias_d = nc.dram_tensor("biasB", [1, 2 * 520 + 1], F32R,
                                kind="ExternalInput")
        ones_d = nc.dram_tensor("ones", [1, PANEL], F32R, kind="ExternalInput")
    res_d = nc.dram_tensor("res", [CHUNK, D], F32, kind="ExternalOutput")

    with tile.TileContext(nc) as tc:
        with (
            tc.tile_pool(name="const", bufs=1) as cp,
            tc.tile_pool(name="wpool", bufs=1) as wp,
            tc.tile_pool(name="ht", bufs=2) as htp,
            tc.tile_pool(name="kvp", bufs=3) as kvp,
            tc.tile_pool(name="memp", bufs=10) as memp,
            tc.tile_pool(name="carry", bufs=1) as carp,
            tc.tile_pool(name="work", bufs=3) as wkp,
            tc.tile_pool(name="zpool", bufs=1) as zp,
            tc.tile_pool(name="io", bufs=2) as iop,
            tc.tile_pool(name="psQ", bufs=4, space="PSUM") as psQ,
            tc.tile_pool(name="psN", bufs=1, space="PSUM") as psN,
            tc.tile_pool(name="psV", bufs=3, space="PSUM") as psV,
        ):
            if has_bias:
                bias = cp.tile([1, 2 * 520 + 1], F32R, tag="bias")
                nc.sync.dma_start(bias[:], bias_d.ap())
                ones = cp.tile([1, PANEL], F32R, tag="ones")
                nc.sync.dma_start(ones[:], ones_d.ap())
            mq = {}
            for nm in ("MqC", "MqS"):
                for dp in range(NDP):
                    t = wp.tile([128, 512], F32R, tag=f"m_{nm}_{dp}")
                    nc.sync.dma_start(
                        t[:], m_d[nm].ap()[dp * 128:(dp + 1) * 128, :])
                    mq[(nm, dp)] = t
            mnyq = []
            for dp in range(NDP):
                t = cp.tile([128, 1], F32R, tag=f"mnyq_{dp}")
                nc.sync.dma_start(t[:], mnyq_d.ap()[dp * 128:(dp + 1) * 128, :])
                mnyq.append(t)
            asb, bsb = [], []
            for ftt in range(FT):
                ta = wp.tile([128, D], F32R, tag=f"a_{ftt}")
                nc.sync.dma_start(ta[:], a_d.ap()[ftt * 128:(ftt + 1) * 128, :])
                asb.append(ta)
                tb = wp.tile([128, D], F32R, tag=f"b_{ftt}")
                nc.sync.dma_start(tb[:], b_d.ap()[ftt * 128:(ftt + 1) * 128, :])
                bsb.append(tb)
            a512 = cp.tile([1, D], F32R, tag="a512")
            nc.sync.dma_start(a512[:], a512_d.ap())

            # scan carries: init columns from DRAM (host prefix)
            carry = []
            for i in range(9):
                t = carp.tile([128, 1], F32, tag=f"car_{i}")
                r0 = i * 128 if i < 8 else 1024
                rows = 128 if i < 8 else 1
                nc.sync.dma_start(t[:rows, :], init_d.ap()[r0:r0 + rows, 0:1])
                carry.append(t)

            for blk in range(NPANEL):
                p0 = blk * PANEL
                ht = [htp.tile([128, PANEL], F32R, tag=f"ht_{dp}",
                               name=f"ht_{blk}_{dp}") for dp in range(NDP)]
                for dp in range(NDP):
                    nc.sync.dma_start(
                        ht[dp][:], htd.ap()[dp * 128:(dp + 1) * 128,
                                            p0:p0 + PANEL])
                # mem for this block: scan kv with chained carry
                mems = []
                for i in range(9):
                    rows = 128 if i < 8 else 1
                    r0 = i * 128 if i < 8 else 1024
                    kvt = kvp.tile([128, PANEL], F32, tag="kvt")
                    nc.sync.dma_start(kvt[:rows, :],
                                      kvd.ap()[r0:r0 + rows, p0:p0 + PANEL])
                    mt = memp.tile([128, PANEL], F32, tag="memt")
                    nc.vector.tensor_tensor_scan(
                        mt[:rows, :], kvt[:rows, :], kvt[:rows, :],
                        carry[i][:rows, :], op0=AT.add, op1=AT.bypass)
                    nc.vector.tensor_copy(carry[i][:rows, :],
                                          mt[:rows, PANEL - 1:PANEL])
                    mems.append(mt)
                zre, zim = [], []
                for ft in range(FT):
                    pq = {}
                    for i, nm in enumerate(("MqC", "MqS")):
                        pt = psQ.tile([128, PANEL], F32, tag="fq")
                        for dp in range(NDP):
                            nc.tensor.matmul(
                                pt[:], mq[(nm, dp)][:, ft * 128:(ft + 1) * 128],
                                ht[dp][:], start=(dp == 0),
                                stop=(dp == NDP - 1 and not has_bias))
                        if has_bias:
                            nc.tensor.matmul(
                                pt[:],
                                bias[:, i * 520 + ft * 128:i * 520 + (ft + 1) * 128],
                                ones[:], start=False, stop=True)
                        pq[nm] = pt
                    t1 = wkp.tile([128, PANEL], F32, tag="t1")
                    t2 = wkp.tile([128, PANEL], F32, tag="t2")
                    zr = zp.tile([128, PANEL], F32R, tag=f"zre_{ft}")
                    zi = zp.tile([128, PANEL], F32R, tag=f"zim_{ft}")
                    nc.vector.tensor_tensor(t1[:], mems[ft][:], pq["MqC"][:],
                                            op=AT.mult)
                    nc.vector.tensor_tensor(t2[:], mems[4 + ft][:],
                                            pq["MqS"][:], op=AT.mult)
                    nc.vector.tensor_tensor(zr[:], t1[:], t2[:],
                                            op=AT.subtract)
                    nc.vector.tensor_tensor(t1[:], mems[ft][:], pq["MqS"][:],
                                            op=AT.mult)
                    nc.vector.tensor_tensor(t2[:], mems[4 + ft][:],
                                            pq["MqC"][:], op=AT.mult)
                    nc.vector.tensor_tensor(zi[:], t1[:], t2[:], op=AT.add)
                    zre.append(zr)
                    zim.append(zi)
                # nyquist fq
                pn = psN.tile([1, PANEL], F32, tag="fqnyq")
                for dp in range(NDP):
                    nc.tensor.matmul(pn[:], mnyq[dp][:], ht[dp][:],
                                     start=(dp == 0),
                                     stop=(dp == NDP - 1 and not has_bias))
                if has_bias:
                    nc.tensor.matmul(pn[:], bias[:, 2 * 520:2 * 520 + 1],
                                     ones[:], start=False, stop=True)
                znyq = zp.tile([1, PANEL], F32R, tag="znyq")
                nc.vector.tensor_tensor(znyq[:], mems[8][0:1, :], pn[:],
                                        op=AT.mult)

                for sub in range(PANEL // 128):
                    ob = iop.tile([128, D], F32, tag="ob")
                    nc.sync.dma_start(
                        ob[:], outp_d.ap()[p0 + sub * 128:p0 + (sub + 1) * 128, :])
                    rs = iop.tile([128, D], F32, tag="rs")
                    s0, s1 = sub * 128, (sub + 1) * 128
                    for half in range(2):
                        pv = psV.tile([128, 512], F32, tag="pv")
                        d0, d1 = half * 512, (half + 1) * 512
                        for ft in range(FT):
                            nc.tensor.matmul(pv[:], zre[ft][:, s0:s1],
                                             asb[ft][:, d0:d1],
                                             start=(ft == 0), stop=False)
                        for ft in range(FT):
                            nc.tensor.matmul(pv[:], zim[ft][:, s0:s1],
                                             bsb[ft][:, d0:d1],
                                             start=False, stop=False)
                        nc.tensor.matmul(pv[:], znyq[:, s0:s1],
                                         a512[:, d0:d1],
                                         start=False, stop=True)
                        nc.vector.tensor_tensor(rs[:, d0:d1], pv[:],
                                                ob[:, d0:d1], op=AT.add)
                    nc.sync.dma_start(
                        res_d.ap()[p0 + sub * 128:p0 + (sub + 1) * 128, :],
                        rs[:])

    _legalize_waits(nc)
    return nc


def _programs(has_bias):
    key = ("ab", has_bias)
    if key not in _cache:
        _cache[key] = (_build_a(has_bias), _build_b(has_bias))
    return _cache[key]


def kernel(output, hidden_states, Wq, bq, Wk, bk, Wv, bv, gate, _trace=False):
    from concourse import bass_utils

    output = np.asarray(output, dtype=np.float32)
    hidden = np.asarray(hidden_states, dtype=np.float32)
    cst = _host_constants(
        np.asarray(Wq, np.float32), np.asarray(bq, np.float32),
        np.asarray(Wk, np.float32), np.asarray(bk, np.float32),
        np.asarray(Wv, np.float32), np.asarray(bv, np.float32),
        np.asarray(gate, np.float32))
    has_bias = bool(np.any(cst["bias"]))
    nca, ncb = _programs(has_bias)

    ac = np.ascontiguousarray
    ident = np.eye(128, dtype=np.float32)
    sharedA = {
        "MkC": ac(cst["MkC"][:, :512]), "MkS": ac(cst["MkS"][:, :512]),
        "MvC": ac(cst["MvC"][:, :512]), "MvS": ac(cst["MvS"][:, :512]),
        "Mnyq": ac(np.stack([cst["MkC"][:, 512], cst["MvC"][:, 512]], axis=1)),
        "identr": ident,
    }
    if has_bias:
        ba = np.zeros((1, 4 * 520 + 2), np.float32)
        for i in range(4):
            ba[0, i * 520:i * 520 + 520] = cst["bias"][i]
        ba[0, 4 * 520 + 0] = cst["bias"][0][512]
        ba[0, 4 * 520 + 1] = cst["bias"][2][512]
        sharedA["biasA"] = ba
        sharedA["ones"] = np.ones((1, PANEL), np.float32)

    chunks = []
    for c in range(NCORES):
        b, j = c // 4, c % 4
        chunks.append((b, j))

    in_a = []
    for (b, j) in chunks:
        im = dict(sharedA)
        im["h"] = ac(hidden[b, j * CHUNK:(j + 1) * CHUNK, :])
        in_a.append(im)
    res_a = bass_utils.run_bass_kernel_spmd(
        nca, in_a, core_ids=list(range(NCORES)), trace=_trace)

    # host: causal prefix over chunk totals
    totals = np.stack([res_a.results[c]["totals"][:, 0] for c in range(NCORES)])
    inits = []
    for c, (b, j) in enumerate(chunks):
        p = np.zeros((1056, 1), np.float32)
        for c2, (b2, j2) in enumerate(chunks):
            if b2 == b and j2 < j:
                p[:, 0] += totals[c2]
        inits.append(p)

    sharedB = {
        "MqC": ac(cst["MqC"][:, :512]), "MqS": ac(cst["MqS"][:, :512]),
        "Mnyq": ac(cst["MqC"][:, 512:513]),
        "A": ac(cst["A"][:512, :]), "Bm": ac(cst["Bm"][:512, :]),
        "A512": ac(cst["A"][512:513, :]),
    }
    if has_bias:
        bb = np.zeros((1, 2 * 520 + 1), np.float32)
        bb[0, 0:520] = cst["bias"][4]
        bb[0, 520:1040] = cst["bias"][5]
        bb[0, 2 * 520] = cst["bias"][4][512]
        sharedB["biasB"] = bb
        sharedB["ones"] = np.ones((1, PANEL), np.float32)

    in_b = []
    for c, (b, j) in enumerate(chunks):
        im = dict(sharedB)
        im["htd"] = res_a.results[c]["htd"]
        im["kvd"] = res_a.results[c]["kvd"]
        im["init"] = inits[c]
        im["outp"] = ac(output[b, j * CHUNK:(j + 1) * CHUNK, :])
        in_b.append(im)
    res_b = bass_utils.run_bass_kernel_spmd(
        ncb, in_b, core_ids=list(range(NCORES)), trace=_trace)

    out = np.empty((B, S, D), dtype=np.float32)
    for c, (b, j) in enumerate(chunks):
        out[b, j * CHUNK:(j + 1) * CHUNK, :] = res_b.results[c]["res"]
    if _trace:
        kernel._last = (res_a, res_b)
    return out



# revision 4
# speedup vs baseline: 1.3180x; 1.3180x over previous
"""Trainium2 Bass kernel for nn_HRRAdaptedAttention (B=2, S=8192, D=1024).

out = output + gate * irfft(cumsum_s(rfft(k)*rfft(v)) * conj(rfft(q))),
q/k/v = hidden @ W.T + b.

Sharding: (batch, seq) -> 8 chunks of 2048 positions, one per core.
The rfft/irfft are folded into the projection weights on the host, so on
device everything is bf16 matmuls, elementwise complex products, and a
per-frequency fp32-state scan over the sequence axis.

Packed spectrum (1024 rows, no separate nyquist work):
  rows 0..511   = C-block: Re coefficients for f = 0..511
  rows 512..1023= S-block: row 512 holds the nyquist (f=512, real) channel
                  in the otherwise-zero S_0 slot; rows 513.. are Im for
                  f = 1..511.
Partition-0 of each S-tile therefore carries f=512; the complex products
need a 2-op fixup per panel for that row (see comments at the fixup sites).

Launch A (per core): fk, fv (bf16 matmuls from host-transposed h^T),
kv = fk*fv, local cumsum (fp32 scan state) -> mem (bf16) to DRAM, plus
fp32 per-row chunk totals via stt accum_out.
Host: exclusive prefix over chunk totals (causal carry across chunks).
Launch B: fq; Z = (mem + carry) * fq via scalar_tensor_tensor (carry
folded into the product op); values = Z @ R (gate/irfft folded into R);
res = output + values.
"""

import numpy as np

B, S, D = 2, 8192, 1024
NCORES = 8
CHUNK = 2048
PANEL = 512
NPANEL = CHUNK // PANEL
NDP = 8                  # 128-row tiles along the contraction (d) axis
NFT = 8                  # 128-row tiles along the packed frequency axis

_cache = {}


def _host_constants(Wq, bq, Wk, bk, Wv, bv, gate):
    import ml_dtypes

    d = np.arange(D, dtype=np.float64)
    f = np.arange(D // 2 + 1, dtype=np.float64)
    ang = 2.0 * np.pi * np.outer(d, f) / D
    C = np.cos(ang)              # [D, 513]
    Sm = -np.sin(ang)

    def fold_pack(W, sign_s=1.0):
        Wt = W.T.astype(np.float64)
        FC = Wt @ C              # [D, 513] Re part
        FS = sign_s * (Wt @ Sm)  # [D, 513] Im part
        P = np.empty((D, D), dtype=np.float64)
        P[:, 0:512] = FC[:, 0:512]
        P[:, 512] = FC[:, 512]          # nyquist -> S-block slot 0
        P[:, 513:1024] = FS[:, 1:512]
        return P

    MkP = fold_pack(Wk)
    MvP = fold_pack(Wv)
    MqP = fold_pack(Wq, sign_s=-1.0)     # conj(fq) folded

    g = float(np.asarray(gate).reshape(-1)[0])
    w = np.full(D // 2 + 1, 2.0)
    w[0] = 1.0
    w[512] = 1.0
    scale = (w * g / D)[:, None]
    A = scale * C.T                      # [513, D] coeff for Z_re
    Bm = scale * Sm.T                    # [513, D] coeff for Z_im
    RP = np.empty((D, D), dtype=np.float64)
    RP[0:512] = A[0:512]
    RP[512] = A[512]                     # nyquist coeff in S-block slot 0
    RP[513:1024] = Bm[1:512]

    def bias_pack(bvec, sign_s=1.0):
        b64 = np.asarray(bvec, np.float64)
        BC = b64 @ C
        BS = sign_s * (b64 @ Sm)
        p = np.empty(D, np.float64)
        p[0:512] = BC[0:512]
        p[512] = BC[512]
        p[513:1024] = BS[1:512]
        return p

    bkP = bias_pack(bk)
    bvP = bias_pack(bv)
    bqP = bias_pack(bq, sign_s=-1.0)

    def tile8(M):
        # [128p, 8192] with block i at cols i*1024..(i+1)*1024, from [1024, 1024]
        return np.ascontiguousarray(
            M.reshape(8, 128, 1024).transpose(1, 0, 2).reshape(128, 8192)
        ).astype(ml_dtypes.bfloat16)

    def col8(v):
        # [1024] -> [128, 8] with row block i in col i
        return np.ascontiguousarray(
            v.reshape(8, 128).T).astype(np.float32)

    return dict(MkP=tile8(MkP), MvP=tile8(MvP), MqP=tile8(MqP), RP=tile8(RP),
                bk=col8(bkP), bv=col8(bvP), bq=col8(bqP),
                has_bias=bool(np.any(bkP) or np.any(bvP) or np.any(bqP)))


_WAIT_EXEMPT = {
    "InstNoOp", "InstEventSemaphore", "InstUnconditionalBranch",
    "InstRegisterMove", "InstCall", "InstISA",
}


def _legalize_waits(nc, max_waits=1):
    """TRN2 instruction structs hold one sync-wait command; move extra waits
    onto same-engine nops inserted just before the instruction."""
    import bass_rust
    import concourse.mybir as mybir
    ctr = 0
    for fn in nc.m.functions:
        for blk in fn.blocks:
            new = []
            for inst in blk.instructions:
                if (type(inst).__name__ not in _WAIT_EXEMPT
                        and inst.sync_info is not None):
                    waits = list(inst.sync_info.on_wait)
                    if len(waits) > max_waits:
                        for w in waits[:-max_waits]:
                            nop = mybir.InstNoOp(
                                name=f"I-lglnop-{ctr}", ins=[], outs=[])
                            ctr += 1
                            nop.engine = inst.engine
                            nop.sync_info = bass_rust.SyncInfo(
                                on_wait=[w], on_update=[])
                            new.append(nop)
                        inst.sync_info = bass_rust.SyncInfo(
                            on_wait=waits[-max_waits:],
                            on_update=inst.sync_info.on_update)
                new.append(inst)
            blk.instructions = new


def _build_a(has_bias):
    import concourse.bass as bass
    import concourse.mybir as mybir
    import concourse.tile as tile
    F32, BF16 = mybir.dt.float32, mybir.dt.bfloat16
    AT = mybir.AluOpType
    AX = mybir.AxisListType.X

    nc = bass.Bass("TRN2", target_bir_lowering=False, debug=False,
                   num_devices=NCORES)
    ht_d = nc.dram_tensor("ht", [128, NDP * CHUNK], BF16, kind="ExternalInput")
    mk_d = nc.dram_tensor("MkP", [128, NDP * 1024], BF16, kind="ExternalInput")
    mv_d = nc.dram_tensor("MvP", [128, NDP * 1024], BF16, kind="ExternalInput")
    if has_bias:
        biask_d = nc.dram_tensor("biask", [128, 8], F32, kind="ExternalInput")
        biasv_d = nc.dram_tensor("biasv", [128, 8], F32, kind="ExternalInput")
    mem_d = nc.dram_tensor("mem", [NPANEL * 128, NFT * PANEL], BF16,
                           kind="ExternalOutput")
    car_d = nc.dram_tensor("car", [128, 8], F32, kind="ExternalOutput")

    with tile.TileContext(nc) as tc:
        with (
            tc.tile_pool(name="const", bufs=1) as cp,
            tc.tile_pool(name="wpool", bufs=1) as wp,
            tc.tile_pool(name="fkv", bufs=2) as fkp,
            tc.tile_pool(name="kv", bufs=2) as kvp,
            tc.tile_pool(name="mem", bufs=2) as memp,
            tc.tile_pool(name="work", bufs=3) as wkp,
            tc.tile_pool(name="redp", bufs=2) as redp,
            tc.tile_pool(name="carp", bufs=1) as carp,
            tc.tile_pool(name="ps", bufs=8, space="PSUM") as psp,
        ):
            mk = wp.tile([128, NDP * 1024], BF16, tag="mk")
            mv = wp.tile([128, NDP * 1024], BF16, tag="mv")
            ht = wp.tile([128, NDP * CHUNK], BF16, tag="ht")
            # stream per-dp so panel-0 matmuls can start early
            for dp in range(NDP):
                nc.sync.dma_start(mk[:, dp * 1024:(dp + 1) * 1024],
                                  mk_d.ap()[:, dp * 1024:(dp + 1) * 1024])
                nc.sync.dma_start(ht[:, dp * CHUNK:dp * CHUNK + CHUNK],
                                  ht_d.ap()[:, dp * CHUNK:dp * CHUNK + CHUNK])
                nc.sync.dma_start(mv[:, dp * 1024:(dp + 1) * 1024],
                                  mv_d.ap()[:, dp * 1024:(dp + 1) * 1024])
            if has_bias:
                bk = cp.tile([128, 8], F32, tag="bk")
                nc.sync.dma_start(bk[:], biask_d.ap())
                bv = cp.tile([128, 8], F32, tag="bv")
                nc.sync.dma_start(bv[:], biasv_d.ap())
            car = carp.tile([128, 8], F32, tag="car")

            for p in range(NPANEL):
                p0 = p * PANEL
                fk = fkp.tile([128, NFT * PANEL], BF16, tag="fk")
                fv = fkp.tile([128, NFT * PANEL], BF16, tag="fv")
                # projections: dp-outer in two groups of 4 freq-tiles so the
                # first panel overlaps the weight/ht streaming
                for (w_t, b_t, out_t) in ((mk, "bk", fk), (mv, "bv", fv)):
                    for g in range(2):
                        ps = [psp.tile([128, PANEL], F32, tag="ps",
                                       name=f"ps_{p}_{b_t}_{g}_{i}")
                              for i in range(4)]
                        for dp in range(NDP):
                            for i, ft in enumerate(range(g * 4, g * 4 + 4)):
                                nc.tensor.matmul(
                                    ps[i][:],
                                    w_t[:, dp * 1024 + ft * 128:
                                        dp * 1024 + (ft + 1) * 128],
                                    ht[:, dp * CHUNK + p0:dp * CHUNK + p0 + PANEL],
                                    start=(dp == 0), stop=(dp == NDP - 1))
                        for i, ft in enumerate(range(g * 4, g * 4 + 4)):
                            sl = out_t[:, ft * PANEL:(ft + 1) * PANEL]
                            if has_bias:
                                bt = bk if b_t == "bk" else bv
                                nc.scalar.activation(
                                    sl, ps[i][:],
                                    mybir.ActivationFunctionType.Identity,
                                    bias=bt[:, ft:ft + 1], scale=1.0)
                            else:
                                nc.scalar.copy(sl, ps[i][:])

                kv = kvp.tile([128, NFT * PANEL], BF16, tag="kv")
                red = redp.tile([128, 8], F32, tag="red")
                for i in range(4):
                    ci = slice(i * PANEL, (i + 1) * PANEL)
                    si = slice((4 + i) * PANEL, (5 + i) * PANEL)
                    u1 = wkp.tile([128, PANEL], BF16, tag="u1")
                    u2 = wkp.tile([128, PANEL], BF16, tag="u2")
                    u3 = wkp.tile([128, PANEL], BF16, tag="u3")
                    u4 = wkp.tile([128, PANEL], BF16, tag="u4")
                    nc.vector.scalar_tensor_tensor(
                        u1[:], fk[:, ci], 1.0, fv[:, ci],
                        op0=AT.mult, op1=AT.mult)
                    nc.vector.scalar_tensor_tensor(
                        u2[:], fk[:, si], 1.0, fv[:, si],
                        op0=AT.mult, op1=AT.mult)
                    nc.vector.scalar_tensor_tensor(
                        u3[:], fk[:, ci], 1.0, fv[:, si],
                        op0=AT.mult, op1=AT.mult)
                    nc.vector.scalar_tensor_tensor(
                        u4[:], fk[:, si], 1.0, fv[:, ci],
                        op0=AT.mult, op1=AT.mult)
                    nc.vector.scalar_tensor_tensor(
                        kv[:, ci], u1[:], 0.0, u2[:],
                        op0=AT.add, op1=AT.subtract,
                        accum_out=red[:, i:i + 1])
                    nc.vector.scalar_tensor_tensor(
                        kv[:, si], u3[:], 0.0, u4[:],
                        op0=AT.add, op1=AT.add,
                        accum_out=red[:, 4 + i:5 + i])
                    if i == 0:
                        # partition 0 of the S-block carries f=512 (nyquist):
                        # kv_re row0 must be the plain DC product u1 (the true
                        # S_0 channel is zero) and the S-slot row0 must carry
                        # the nyquist product u2 so the scan covers it.
                        nc.vector.tensor_copy(kv[0:1, ci], u1[0:1, :])
                        nc.vector.tensor_copy(kv[0:1, si], u2[0:1, :])
                        rn = wkp.tile([1, 1], F32, tag="rn")
                        nc.vector.tensor_reduce(rn[:], u2[0:1, :],
                                                axis=AX, op=AT.add)
                        nc.vector.tensor_tensor(red[0:1, 0:1], red[0:1, 0:1],
                                                rn[:], op=AT.add)
                        nc.vector.tensor_copy(red[0:1, 4:5], rn[:])

                mem = memp.tile([128, NFT * PANEL], BF16, tag="mem")
                for ft in range(NFT):
                    sl = slice(ft * PANEL, (ft + 1) * PANEL)
                    nc.vector.tensor_tensor_scan(
                        mem[:, sl], kv[:, sl], kv[:, sl],
                        0.0 if p == 0 else car[:, ft:ft + 1],
                        op0=AT.add, op1=AT.bypass)
                if p == 0:
                    nc.gpsimd.tensor_copy(car[:], red[:])
                else:
                    nc.gpsimd.tensor_tensor(car[:], car[:], red[:], op=AT.add)
                nc.sync.dma_start(mem_d.ap()[p * 128:(p + 1) * 128, :], mem[:])

            nc.sync.dma_start(car_d.ap(), car[:])

    _legalize_waits(nc)
    return nc


def _build_b(has_bias):
    import concourse.bass as bass
    import concourse.mybir as mybir
    import concourse.tile as tile
    F32, BF16 = mybir.dt.float32, mybir.dt.bfloat16
    AT = mybir.AluOpType

    nc = bass.Bass("TRN2", target_bir_lowering=False, debug=False,
                   num_devices=NCORES)
    ht_d = nc.dram_tensor("ht", [128, NDP * CHUNK], BF16, kind="ExternalInput")
    mq_d = nc.dram_tensor("MqP", [128, NDP * 1024], BF16, kind="ExternalInput")
    r_d = nc.dram_tensor("RP", [128, NFT * 1024], BF16, kind="ExternalInput")
    mem_d = nc.dram_tensor("mem", [NPANEL * 128, NFT * PANEL], BF16,
                           kind="ExternalInput")
    init_d = nc.dram_tensor("init", [128, 8], F32, kind="ExternalInput")
    outp_d = nc.dram_tensor("outp", [CHUNK, D], F32, kind="ExternalInput")
    if has_bias:
        biasq_d = nc.dram_tensor("biasq", [128, 8], F32, kind="ExternalInput")
    res_d = nc.dram_tensor("res", [CHUNK, D], F32, kind="ExternalOutput")

    with tile.TileContext(nc) as tc:
        with (
            tc.tile_pool(name="const", bufs=1) as cp,
            tc.tile_pool(name="wpool", bufs=1) as wp,
            tc.tile_pool(name="qf", bufs=2) as qfp,
            tc.tile_pool(name="z", bufs=2) as zp,
            tc.tile_pool(name="mem", bufs=2) as memp,
            tc.tile_pool(name="work", bufs=3) as wkp,
            tc.tile_pool(name="io", bufs=2) as iop,
            tc.tile_pool(name="psQ", bufs=4, space="PSUM") as psQ,
            tc.tile_pool(name="psV", bufs=4, space="PSUM") as psV,
        ):
            mq = wp.tile([128, NDP * 1024], BF16, tag="mq")
            ht = wp.tile([128, NDP * CHUNK], BF16, tag="ht")
            rp = wp.tile([128, NFT * 1024], BF16, tag="rp")
            for dp in range(NDP):
                nc.sync.dma_start(mq[:, dp * 1024:(dp + 1) * 1024],
                                  mq_d.ap()[:, dp * 1024:(dp + 1) * 1024])
                nc.sync.dma_start(ht[:, dp * CHUNK:dp * CHUNK + CHUNK],
                                  ht_d.ap()[:, dp * CHUNK:dp * CHUNK + CHUNK])
            car = cp.tile([128, 8], F32, tag="car")
            nc.sync.dma_start(car[:], init_d.ap())
            for rt in range(NFT):
                nc.sync.dma_start(rp[:, rt * 1024:(rt + 1) * 1024],
                                  r_d.ap()[:, rt * 1024:(rt + 1) * 1024])
            if has_bias:
                bq = cp.tile([128, 8], F32, tag="bq")
                nc.sync.dma_start(bq[:], biasq_d.ap())

            mems = []
            for p in range(NPANEL):
                m = memp.tile([128, NFT * PANEL], BF16, tag="mem",
                              name=f"mem_{p}")
                mems.append(m)
            nc.sync.dma_start(mems[0][:], mem_d.ap()[0:128, :])
            obs = {}

            def emit_q(p):
                p0 = p * PANEL
                qf = qfp.tile([128, NFT * PANEL], BF16, tag="qf",
                              name=f"qf_{p}")
                for g in range(2):
                    ps = [psQ.tile([128, PANEL], F32, tag="psq",
                                   name=f"psq_{p}_{g}_{i}")
                          for i in range(4)]
                    for dp in range(NDP):
                        for i, ft in enumerate(range(g * 4, g * 4 + 4)):
                            nc.tensor.matmul(
                                ps[i][:],
                                mq[:, dp * 1024 + ft * 128:
                                   dp * 1024 + (ft + 1) * 128],
                                ht[:, dp * CHUNK + p0:dp * CHUNK + p0 + PANEL],
                                start=(dp == 0), stop=(dp == NDP - 1))
                    for i, ft in enumerate(range(g * 4, g * 4 + 4)):
                        sl = qf[:, ft * PANEL:(ft + 1) * PANEL]
                        if has_bias:
                            nc.scalar.activation(
                                sl, ps[i][:],
                                mybir.ActivationFunctionType.Identity,
                                bias=bq[:, ft:ft + 1], scale=1.0)
                        else:
                            nc.scalar.copy(sl, ps[i][:])
                # prefetch next panel's mem and this panel's output rows
                if p + 1 < NPANEL:
                    nc.sync.dma_start(mems[p + 1][:],
                                      mem_d.ap()[(p + 1) * 128:(p + 2) * 128, :])
                obl = []
                for sub in range(PANEL // 128):
                    ob = iop.tile([128, D], F32, tag="ob",
                                  name=f"ob_{p}_{sub}")
                    nc.sync.dma_start(
                        ob[:],
                        outp_d.ap()[p0 + sub * 128:p0 + (sub + 1) * 128, :])
                    obl.append(ob)
                obs[p] = obl

                # Z = (mem + carry) * fq, carry folded into the stt ops
                zc = zp.tile([128, 4 * PANEL], BF16, tag="zc", name=f"zc_{p}")
                zs = zp.tile([128, 4 * PANEL], BF16, tag="zs", name=f"zs_{p}")
                mem = mems[p]
                for i in range(4):
                    ci = slice(i * PANEL, (i + 1) * PANEL)
                    si = slice((4 + i) * PANEL, (5 + i) * PANEL)
                    u1 = wkp.tile([128, PANEL], BF16, tag="u1")
                    u2 = wkp.tile([128, PANEL], BF16, tag="u2")
                    u3 = wkp.tile([128, PANEL], BF16, tag="u3")
                    u4 = wkp.tile([128, PANEL], BF16, tag="u4")
                    nc.vector.scalar_tensor_tensor(
                        u1[:], mem[:, ci], car[:, i:i + 1], qf[:, ci],
                        op0=AT.add, op1=AT.mult)
                    nc.vector.scalar_tensor_tensor(
                        u2[:], mem[:, si], car[:, 4 + i:5 + i], qf[:, si],
                        op0=AT.add, op1=AT.mult)
                    nc.vector.scalar_tensor_tensor(
                        u3[:], mem[:, ci], car[:, i:i + 1], qf[:, si],
                        op0=AT.add, op1=AT.mult)
                    nc.vector.scalar_tensor_tensor(
                        u4[:], mem[:, si], car[:, 4 + i:5 + i], qf[:, ci],
                        op0=AT.add, op1=AT.mult)
                    nc.vector.tensor_tensor(zc[:, ci], u1[:], u2[:],
                                            op=AT.subtract)
                    nc.vector.tensor_tensor(zs[:, ci], u3[:], u4[:],
                                            op=AT.add)
                    if i == 0:
                        # S-block partition 0 is the nyquist channel: Z_re
                        # row0 is the plain DC product u1 and the S-slot
                        # row0 must carry Z_512 = u2 (R row 512 holds A512).
                        nc.vector.tensor_copy(zc[0:1, ci], u1[0:1, :])
                        nc.vector.tensor_copy(zs[0:1, ci], u2[0:1, :])
                return zc, zs

            def emit_v(p, zcs):
                p0 = p * PANEL
                zc, zs = zcs
                for sub in range(PANEL // 128):
                    ob = obs[p][sub]
                    rs = iop.tile([128, D], F32, tag="rs")
                    s0, s1 = sub * 128, (sub + 1) * 128
                    for half in range(2):
                        pv = psV.tile([128, 512], F32, tag="pv")
                        d0 = half * 512
                        for i in range(4):
                            nc.tensor.matmul(
                                pv[:], zc[:, i * PANEL + s0:i * PANEL + s1],
                                rp[:, i * 1024 + d0:i * 1024 + d0 + 512],
                                start=(i == 0), stop=False)
                        for i in range(4):
                            nc.tensor.matmul(
                                pv[:], zs[:, i * PANEL + s0:i * PANEL + s1],
                                rp[:, (4 + i) * 1024 + d0:
                                   (4 + i) * 1024 + d0 + 512],
                                start=False, stop=(i == 3))
                        nc.vector.tensor_tensor(rs[:, d0:d0 + 512], pv[:],
                                                ob[:, d0:d0 + 512], op=AT.add)
                    nc.sync.dma_start(
                        res_d.ap()[p0 + sub * 128:p0 + (sub + 1) * 128, :],
                        rs[:])

            # software pipeline: PE does q(p+1) while DVE/Act build Z(p)
            z0 = emit_q(0)
            z1 = emit_q(1)
            emit_v(0, z0)
            z2 = emit_q(2)
            emit_v(1, z1)
            z3 = emit_q(3)
            emit_v(2, z2)
            emit_v(3, z3)

    _legalize_waits(nc)
    return nc


def _programs(has_bias):
    key = ("ab", has_bias)
    if key not in _cache:
        _cache[key] = (_build_a(has_bias), _build_b(has_bias))
    return _cache[key]


def kernel(output, hidden_states, Wq, bq, Wk, bk, Wv, bv, gate, _trace=False):
    import ml_dtypes
    from concourse import bass_utils

    output = np.asarray(output, dtype=np.float32)
    hidden = np.asarray(hidden_states, dtype=np.float32)
    cst = _host_constants(
        np.asarray(Wq, np.float32), np.asarray(bq, np.float32),
        np.asarray(Wk, np.float32), np.asarray(bk, np.float32),
        np.asarray(Wv, np.float32), np.asarray(bv, np.float32),
        np.asarray(gate, np.float32))
    has_bias = cst["has_bias"]
    nca, ncb = _programs(has_bias)

    ac = np.ascontiguousarray
    chunks = [(c // 4, c % 4) for c in range(NCORES)]

    def ht_pack(b, j):
        hT = hidden[b, j * CHUNK:(j + 1) * CHUNK, :].T  # [1024, 2048]
        return ac(hT.reshape(8, 128, CHUNK).transpose(1, 0, 2)
                  .reshape(128, 8 * CHUNK)).astype(ml_dtypes.bfloat16)

    sharedA = {"MkP": cst["MkP"], "MvP": cst["MvP"]}
    if has_bias:
        sharedA["biask"] = cst["bk"]
        sharedA["biasv"] = cst["bv"]

    hts = [ht_pack(b, j) for (b, j) in chunks]
    in_a = []
    for c, (b, j) in enumerate(chunks):
        im = dict(sharedA)
        im["ht"] = hts[c]
        in_a.append(im)
    res_a = bass_utils.run_bass_kernel_spmd(
        nca, in_a, core_ids=list(range(NCORES)), trace=_trace)

    # host: causal prefix over per-chunk totals (fp32)
    cars = [np.asarray(res_a.results[c]["car"], np.float32)
            for c in range(NCORES)]
    inits = []
    for c, (b, j) in enumerate(chunks):
        p = np.zeros((128, 8), np.float32)
        for c2, (b2, j2) in enumerate(chunks):
            if b2 == b and j2 < j:
                p += cars[c2]
        inits.append(p)

    sharedB = {"MqP": cst["MqP"], "RP": cst["RP"]}
    if has_bias:
        sharedB["biasq"] = cst["bq"]

    in_b = []
    for c, (b, j) in enumerate(chunks):
        im = dict(sharedB)
        im["ht"] = hts[c]
        im["mem"] = res_a.results[c]["mem"]
        im["init"] = inits[c]
        im["outp"] = ac(output[b, j * CHUNK:(j + 1) * CHUNK, :])
        in_b.append(im)
    res_b = bass_utils.run_bass_kernel_spmd(
        ncb, in_b, core_ids=list(range(NCORES)), trace=_trace)

    out = np.empty((B, S, D), dtype=np.float32)
    for c, (b, j) in enumerate(chunks):
        out[b, j * CHUNK:(j + 1) * CHUNK, :] = res_b.results[c]["res"]
    if _trace:
        kernel._last = (res_a, res_b)
    return out


# revision 6
# speedup vs baseline: 1.4336x; 1.0877x over previous
"""Trainium2 Bass kernel for nn_HRRAdaptedAttention (B=2, S=8192, D=1024).

out = output + gate * irfft(cumsum_s(rfft(k)*rfft(v)) * conj(rfft(q))),
q/k/v = hidden @ W.T + b.

Sharding: (batch, seq) -> 8 chunks of 2048 positions, one per core.
The rfft/irfft are folded into the projection weights on the host, so on
device everything is bf16 matmuls, elementwise complex products, and a
per-frequency fp32-state scan over the sequence axis.

Packed spectrum (1024 rows, no separate nyquist matmuls):
  rows 0..511    = C-block: Re coefficients for f = 0..511
  rows 512..1023 = S-block: row 512 holds the nyquist (f=512, real)
                   channel in the otherwise-zero S_0 slot; rows 513.. are
                   Im for f = 1..511.
Partition 0 of each S-tile therefore carries f=512, which needs a few
single-partition fixups per panel (see comments at the fixup sites).

Launch A (per core): fk, fv (bf16 matmuls from host-transposed h^T);
the complex product's combine step is fused into the cumsum scan
(state = (u1 + state) - u2), mem (bf16) to DRAM; chunk totals are the
scan carries.  Host: exclusive prefix over chunk totals.
Launch B: fq; Z = (mem + carry) * fq with the carry folded in on the
Act engine (Identity + per-partition bias); values = Z @ R (gate/irfft
folded into R); res = output + values.
"""

import numpy as np

B, S, D = 2, 8192, 1024
NCORES = 8
CHUNK = 2048
PANEL = 512
NPANEL = CHUNK // PANEL
NDP = 8                  # 128-row tiles along the contraction (d) axis
NFT = 8                  # 128-row tiles along the packed frequency axis

_cache = {}


def _host_constants(Wq, bq, Wk, bk, Wv, bv, gate):
    import ml_dtypes

    d = np.arange(D, dtype=np.float64)
    f = np.arange(D // 2 + 1, dtype=np.float64)
    ang = 2.0 * np.pi * np.outer(d, f) / D
    C = np.cos(ang)              # [D, 513]
    Sm = -np.sin(ang)

    def fold_pack(W, sign_s=1.0):
        Wt = W.T.astype(np.float64)
        FC = Wt @ C              # [D, 513] Re part
        FS = sign_s * (Wt @ Sm)  # [D, 513] Im part
        P = np.empty((D, D), dtype=np.float64)
        P[:, 0:512] = FC[:, 0:512]
        P[:, 512] = FC[:, 512]          # nyquist -> S-block slot 0
        P[:, 513:1024] = FS[:, 1:512]
        return P

    MkP = fold_pack(Wk)
    MvP = fold_pack(Wv)
    MqP = fold_pack(Wq, sign_s=-1.0)     # conj(fq) folded

    g = float(np.asarray(gate).reshape(-1)[0])
    w = np.full(D // 2 + 1, 2.0)
    w[0] = 1.0
    w[512] = 1.0
    scale = (w * g / D)[:, None]
    A = scale * C.T                      # [513, D] coeff for Z_re
    Bm = scale * Sm.T                    # [513, D] coeff for Z_im
    RP = np.empty((D, D), dtype=np.float64)
    RP[0:512] = A[0:512]
    RP[512] = A[512]                     # nyquist coeff in S-block slot 0
    RP[513:1024] = Bm[1:512]

    def bias_pack(bvec, sign_s=1.0):
        b64 = np.asarray(bvec, np.float64)
        BC = b64 @ C
        BS = sign_s * (b64 @ Sm)
        p = np.empty(D, np.float64)
        p[0:512] = BC[0:512]
        p[512] = BC[512]
        p[513:1024] = BS[1:512]
        return p

    bkP = bias_pack(bk)
    bvP = bias_pack(bv)
    bqP = bias_pack(bq, sign_s=-1.0)

    def tile8(M):
        # [128p, 8192] with block i at cols i*1024..(i+1)*1024, from [1024, 1024]
        return np.ascontiguousarray(
            M.reshape(8, 128, 1024).transpose(1, 0, 2).reshape(128, 8192)
        ).astype(ml_dtypes.bfloat16)

    def col8(v):
        # [1024] -> [128, 8] with row block i in col i
        return np.ascontiguousarray(
            v.reshape(8, 128).T).astype(np.float32)

    return dict(MkP=tile8(MkP), MvP=tile8(MvP), MqP=tile8(MqP), RP=tile8(RP),
                bk=col8(bkP), bv=col8(bvP), bq=col8(bqP),
                has_bias=bool(np.any(bkP) or np.any(bvP) or np.any(bqP)))


_WAIT_EXEMPT = {
    "InstNoOp", "InstEventSemaphore", "InstUnconditionalBranch",
    "InstRegisterMove", "InstCall", "InstISA",
}


def _legalize_waits(nc, max_waits=1):
    """TRN2 instruction structs hold one sync-wait command; move extra waits
    onto same-engine nops inserted just before the instruction."""
    import bass_rust
    import concourse.mybir as mybir
    ctr = 0
    for fn in nc.m.functions:
        for blk in fn.blocks:
            new = []
            for inst in blk.instructions:
                if (type(inst).__name__ not in _WAIT_EXEMPT
                        and inst.sync_info is not None):
                    waits = list(inst.sync_info.on_wait)
                    if len(waits) > max_waits:
                        for w in waits[:-max_waits]:
                            nop = mybir.InstNoOp(
                                name=f"I-lglnop-{ctr}", ins=[], outs=[])
                            ctr += 1
                            nop.engine = inst.engine
                            nop.sync_info = bass_rust.SyncInfo(
                                on_wait=[w], on_update=[])
                            new.append(nop)
                        inst.sync_info = bass_rust.SyncInfo(
                            on_wait=waits[-max_waits:],
                            on_update=inst.sync_info.on_update)
                new.append(inst)
            blk.instructions = new


def _build_a(has_bias):
    import concourse.bass as bass
    import concourse.mybir as mybir
    import concourse.tile as tile
    F32, BF16 = mybir.dt.float32, mybir.dt.bfloat16
    AT = mybir.AluOpType
    AF = mybir.ActivationFunctionType

    nc = bass.Bass("TRN2", target_bir_lowering=False, debug=False,
                   num_devices=NCORES)
    ht_d = nc.dram_tensor("ht", [128, NDP * CHUNK], BF16, kind="ExternalInput")
    mk_d = nc.dram_tensor("MkP", [128, NDP * 1024], BF16, kind="ExternalInput")
    mv_d = nc.dram_tensor("MvP", [128, NDP * 1024], BF16, kind="ExternalInput")
    if has_bias:
        biask_d = nc.dram_tensor("biask", [128, 8], F32, kind="ExternalInput")
        biasv_d = nc.dram_tensor("biasv", [128, 8], F32, kind="ExternalInput")
    mem_d = nc.dram_tensor("mem", [NPANEL * 128, NFT * PANEL], BF16,
                           kind="ExternalOutput")
    car_d = nc.dram_tensor("car", [128, 8], F32, kind="ExternalOutput")

    with tile.TileContext(nc) as tc:
        with (
            tc.tile_pool(name="const", bufs=1) as cp,
            tc.tile_pool(name="wpool", bufs=1) as wp,
            tc.tile_pool(name="fkv", bufs=2) as fkp,
            tc.tile_pool(name="mem", bufs=2) as memp,
            tc.tile_pool(name="work", bufs=3) as wkp,
            tc.tile_pool(name="carp", bufs=1) as carp,
            tc.tile_pool(name="ps", bufs=8, space="PSUM") as psp,
        ):
            mk = wp.tile([128, NDP * 1024], BF16, tag="mk")
            mv = wp.tile([128, NDP * 1024], BF16, tag="mv")
            ht = wp.tile([128, NDP * CHUNK], BF16, tag="ht")
            # stream per-dp so panel-0 matmuls can start early
            for dp in range(NDP):
                nc.sync.dma_start(mk[:, dp * 1024:(dp + 1) * 1024],
                                  mk_d.ap()[:, dp * 1024:(dp + 1) * 1024])
                nc.sync.dma_start(mv[:, dp * 1024:(dp + 1) * 1024],
                                  mv_d.ap()[:, dp * 1024:(dp + 1) * 1024])
                nc.sync.dma_start(ht[:, dp * CHUNK:dp * CHUNK + CHUNK],
                                  ht_d.ap()[:, dp * CHUNK:dp * CHUNK + CHUNK])
            if has_bias:
                bk = cp.tile([128, 8], F32, tag="bk")
                nc.sync.dma_start(bk[:], biask_d.ap())
                bv = cp.tile([128, 8], F32, tag="bv")
                nc.sync.dma_start(bv[:], biasv_d.ap())
            car = carp.tile([128, 8], F32, tag="car")

            for p in range(NPANEL):
                p0 = p * PANEL
                mem = memp.tile([128, NFT * PANEL], BF16, tag="mem",
                                name=f"mem_{p}")
                # two double-pair groups per panel; each uses all 8 PSUM
                # banks with dp-outer accumulation so panel 0 overlaps the
                # weight/ht streaming
                for g in range(2):
                    pr0 = g * 2
                    fts = [pr0, 4 + pr0, pr0 + 1, 4 + pr0 + 1]
                    ps = {}
                    for wt, wnm in ((mk, "k"), (mv, "v")):
                        for ft in fts:
                            ps[(wnm, ft)] = psp.tile(
                                [128, PANEL], F32, tag="ps",
                                name=f"ps_{p}_{g}_{wnm}_{ft}")
                    for dp in range(NDP):
                        for wt, wnm in ((mk, "k"), (mv, "v")):
                            for ft in fts:
                                nc.tensor.matmul(
                                    ps[(wnm, ft)][:],
                                    wt[:, dp * 1024 + ft * 128:
                                       dp * 1024 + (ft + 1) * 128],
                                    ht[:, dp * CHUNK + p0:
                                       dp * CHUNK + p0 + PANEL],
                                    start=(dp == 0), stop=(dp == NDP - 1))
                    fkv = {}
                    for wt, wnm, bnm in ((mk, "k", "bk"), (mv, "v", "bv")):
                        for ft in fts:
                            t = fkp.tile([128, PANEL], BF16,
                                         tag=f"f_{wnm}_{ft % 2}_{ft // 4}",
                                         name=f"f_{p}_{wnm}_{ft}")
                            if has_bias:
                                bt = bk if bnm == "bk" else bv
                                nc.scalar.activation(
                                    t[:], ps[(wnm, ft)][:], AF.Identity,
                                    bias=bt[:, ft:ft + 1], scale=1.0)
                            else:
                                nc.scalar.copy(t[:], ps[(wnm, ft)][:])
                            fkv[(wnm, ft)] = t

                    for i in (pr0, pr0 + 1):
                        u1 = wkp.tile([128, PANEL], BF16, tag="u1")
                        u2 = wkp.tile([128, PANEL], BF16, tag="u2")
                        u3 = wkp.tile([128, PANEL], BF16, tag="u3")
                        u4 = wkp.tile([128, PANEL], BF16, tag="u4")
                        nc.vector.tensor_tensor(
                            u1[:], fkv[("k", i)][:], fkv[("v", i)][:],
                            op=AT.mult)
                        nc.vector.tensor_tensor(
                            u2[:], fkv[("k", 4 + i)][:], fkv[("v", 4 + i)][:],
                            op=AT.mult)
                        nc.vector.tensor_tensor(
                            u3[:], fkv[("k", i)][:], fkv[("v", 4 + i)][:],
                            op=AT.mult)
                        nc.vector.tensor_tensor(
                            u4[:], fkv[("k", 4 + i)][:], fkv[("v", i)][:],
                            op=AT.mult)
                        ci = slice(i * PANEL, (i + 1) * PANEL)
                        si = slice((4 + i) * PANEL, (5 + i) * PANEL)
                        # combine fused into the scan: state=(u1+state)-u2
                        nc.vector.tensor_tensor_scan(
                            mem[:, ci], u1[:], u2[:],
                            0.0 if p == 0 else car[:, i:i + 1],
                            op0=AT.add, op1=AT.subtract)
                        nc.vector.tensor_tensor_scan(
                            mem[:, si], u3[:], u4[:],
                            0.0 if p == 0 else car[:, 4 + i:5 + i],
                            op0=AT.add, op1=AT.add)
                        if i == 0:
                            # partition 0 of the S-block is the nyquist
                            # channel: row 0 of the C-scan must not have
                            # subtracted u2 (true S_0 contribution is zero)
                            # and row 0 of the S-scan must be the cumsum of
                            # the nyquist product u2 itself.
                            ny = wkp.tile([1, PANEL], F32, tag="ny")
                            nc.vector.tensor_tensor_scan(
                                ny[:], u2[0:1, :], u2[0:1, :], 0.0,
                                op0=AT.add, op1=AT.bypass)
                            nc.vector.tensor_tensor(
                                mem[0:1, ci], mem[0:1, ci], ny[:], op=AT.add)
                            nc.vector.tensor_scalar(
                                mem[0:1, si], ny[:],
                                0.0 if p == 0 else car[0:1, 4:5], None,
                                op0=AT.add)
                        # chunk-total carries = scan state at panel end
                        nc.vector.tensor_copy(car[:, i:i + 1],
                                              mem[:, (i + 1) * PANEL - 1:
                                                  (i + 1) * PANEL])
                        nc.vector.tensor_copy(car[:, 4 + i:5 + i],
                                              mem[:, (5 + i) * PANEL - 1:
                                                  (5 + i) * PANEL])
                        nc.sync.dma_start(
                            mem_d.ap()[p * 128:(p + 1) * 128,
                                       i * PANEL:(i + 1) * PANEL],
                            mem[:, ci])
                        nc.sync.dma_start(
                            mem_d.ap()[p * 128:(p + 1) * 128,
                                       (4 + i) * PANEL:(5 + i) * PANEL],
                            mem[:, si])

            nc.sync.dma_start(car_d.ap(), car[:])

    _legalize_waits(nc)
    return nc


def _build_b(has_bias):
    import concourse.bass as bass
    import concourse.mybir as mybir
    import concourse.tile as tile
    F32, BF16 = mybir.dt.float32, mybir.dt.bfloat16
    AT = mybir.AluOpType
    AF = mybir.ActivationFunctionType

    nc = bass.Bass("TRN2", target_bir_lowering=False, debug=False,
                   num_devices=NCORES)
    ht_d = nc.dram_tensor("ht", [128, NDP * CHUNK], BF16, kind="ExternalInput")
    mq_d = nc.dram_tensor("MqP", [128, NDP * 1024], BF16, kind="ExternalInput")
    r_d = nc.dram_tensor("RP", [128, NFT * 1024], BF16, kind="ExternalInput")
    mem_d = nc.dram_tensor("mem", [NPANEL * 128, NFT * PANEL], BF16,
                           kind="ExternalInput")
    init_d = nc.dram_tensor("init", [128, 8], F32, kind="ExternalInput")
    outp_d = nc.dram_tensor("outp", [CHUNK, D], F32, kind="ExternalInput")
    if has_bias:
        biasq_d = nc.dram_tensor("biasq", [128, 8], F32, kind="ExternalInput")
    res_d = nc.dram_tensor("res", [CHUNK, D], F32, kind="ExternalOutput")

    with tile.TileContext(nc) as tc:
        with (
            tc.tile_pool(name="const", bufs=1) as cp,
            tc.tile_pool(name="wpool", bufs=1) as wp,
            tc.tile_pool(name="qf", bufs=2) as qfp,
            tc.tile_pool(name="mp", bufs=2) as mpp,
            tc.tile_pool(name="z", bufs=2) as zp,
            tc.tile_pool(name="mem", bufs=2) as memp,
            tc.tile_pool(name="work", bufs=3) as wkp,
            tc.tile_pool(name="io", bufs=2) as iop,
            tc.tile_pool(name="ps", bufs=8, space="PSUM") as psp,
        ):
            mq = wp.tile([128, NDP * 1024], BF16, tag="mq")
            ht = wp.tile([128, NDP * CHUNK], BF16, tag="ht")
            rp = wp.tile([128, NFT * 1024], BF16, tag="rp")
            for dp in range(NDP):
                nc.sync.dma_start(mq[:, dp * 1024:(dp + 1) * 1024],
                                  mq_d.ap()[:, dp * 1024:(dp + 1) * 1024])
                nc.sync.dma_start(ht[:, dp * CHUNK:dp * CHUNK + CHUNK],
                                  ht_d.ap()[:, dp * CHUNK:dp * CHUNK + CHUNK])
            car = cp.tile([128, 8], F32, tag="car")
            nc.sync.dma_start(car[:], init_d.ap())
            for rt in range(NFT):
                nc.sync.dma_start(rp[:, rt * 1024:(rt + 1) * 1024],
                                  r_d.ap()[:, rt * 1024:(rt + 1) * 1024])
            if has_bias:
                bq = cp.tile([128, 8], F32, tag="bq")
                nc.sync.dma_start(bq[:], biasq_d.ap())

            mems = []
            for p in range(NPANEL):
                m = memp.tile([128, NFT * PANEL], BF16, tag="mem",
                              name=f"mem_{p}")
                mems.append(m)
            nc.sync.dma_start(mems[0][:], mem_d.ap()[0:128, :])
            obs = {}

            def emit_q(p):
                p0 = p * PANEL
                qf = qfp.tile([128, NFT * PANEL], BF16, tag="qf",
                              name=f"qf_{p}")
                groups = [range(8)] if p == 0 else [range(4), range(4, 8)]
                for grp in groups:
                    ps = {ft: psp.tile([128, PANEL], F32, tag="ps",
                                       name=f"psq_{p}_{ft}")
                          for ft in grp}
                    for dp in range(NDP):
                        for ft in grp:
                            nc.tensor.matmul(
                                ps[ft][:],
                                mq[:, dp * 1024 + ft * 128:
                                   dp * 1024 + (ft + 1) * 128],
                                ht[:, dp * CHUNK + p0:dp * CHUNK + p0 + PANEL],
                                start=(dp == 0), stop=(dp == NDP - 1))
                    for ft in grp:
                        sl = qf[:, ft * PANEL:(ft + 1) * PANEL]
                        if has_bias:
                            nc.scalar.activation(
                                sl, ps[ft][:], AF.Identity,
                                bias=bq[:, ft:ft + 1], scale=1.0)
                        else:
                            nc.scalar.copy(sl, ps[ft][:])
                # prefetch next panel's mem and this panel's output rows
                if p + 1 < NPANEL:
                    nc.sync.dma_start(mems[p + 1][:],
                                      mem_d.ap()[(p + 1) * 128:(p + 2) * 128, :])
                obl = []
                for sub in range(PANEL // 128):
                    ob = iop.tile([128, D], F32, tag="ob",
                                  name=f"ob_{p}_{sub}")
                    nc.sync.dma_start(
                        ob[:],
                        outp_d.ap()[p0 + sub * 128:p0 + (sub + 1) * 128, :])
                    obl.append(ob)
                obs[p] = obl

                # memP = mem + carry on the Act engine (Identity + bias)
                mem = mems[p]
                mp = mpp.tile([128, NFT * PANEL], BF16, tag="mp",
                              name=f"mp_{p}")
                for ft in range(NFT):
                    sl = slice(ft * PANEL, (ft + 1) * PANEL)
                    nc.scalar.activation(mp[:, sl], mem[:, sl], AF.Identity,
                                         bias=car[:, ft:ft + 1], scale=1.0)

                zc = zp.tile([128, 4 * PANEL], BF16, tag="zc", name=f"zc_{p}")
                zs = zp.tile([128, 4 * PANEL], BF16, tag="zs", name=f"zs_{p}")
                for i in range(4):
                    ci = slice(i * PANEL, (i + 1) * PANEL)
                    si = slice((4 + i) * PANEL, (5 + i) * PANEL)
                    u1 = wkp.tile([128, PANEL], BF16, tag="u1")
                    u2 = wkp.tile([128, PANEL], BF16, tag="u2")
                    u3 = wkp.tile([128, PANEL], BF16, tag="u3")
                    u4 = wkp.tile([128, PANEL], BF16, tag="u4")
                    nc.vector.tensor_tensor(u1[:], mp[:, ci], qf[:, ci],
                                            op=AT.mult)
                    nc.vector.tensor_tensor(u2[:], mp[:, si], qf[:, si],
                                            op=AT.mult)
                    nc.vector.tensor_tensor(u3[:], mp[:, ci], qf[:, si],
                                            op=AT.mult)
                    nc.vector.tensor_tensor(u4[:], mp[:, si], qf[:, ci],
                                            op=AT.mult)
                    nc.vector.tensor_tensor(zc[:, ci], u1[:], u2[:],
                                            op=AT.subtract)
                    nc.vector.tensor_tensor(zs[:, ci], u3[:], u4[:],
                                            op=AT.add)
                    if i == 0:
                        # S-block partition 0 is the nyquist channel: Z_re
                        # row0 is the plain DC product u1 and the S-slot
                        # row0 carries Z_512 = u2 (R row 512 holds A512).
                        nc.vector.tensor_copy(zc[0:1, ci], u1[0:1, :])
                        nc.vector.tensor_copy(zs[0:1, ci], u2[0:1, :])
                return zc, zs

            def emit_v(p, zcs):
                p0 = p * PANEL
                zc, zs = zcs
                for sub in range(PANEL // 128):
                    ob = obs[p][sub]
                    rs = iop.tile([128, D], F32, tag="rs")
                    s0, s1 = sub * 128, (sub + 1) * 128
                    for half in range(2):
                        pv = psp.tile([128, 512], F32, tag="ps",
                                      name=f"pv_{p}_{sub}_{half}")
                        d0 = half * 512
                        for i in range(4):
                            nc.tensor.matmul(
                                pv[:], zc[:, i * PANEL + s0:i * PANEL + s1],
                                rp[:, i * 1024 + d0:i * 1024 + d0 + 512],
                                start=(i == 0), stop=False)
                        for i in range(4):
                            nc.tensor.matmul(
                                pv[:], zs[:, i * PANEL + s0:i * PANEL + s1],
                                rp[:, (4 + i) * 1024 + d0:
                                   (4 + i) * 1024 + d0 + 512],
                                start=False, stop=(i == 3))
                        nc.vector.tensor_tensor(rs[:, d0:d0 + 512], pv[:],
                                                ob[:, d0:d0 + 512], op=AT.add)
                    nc.sync.dma_start(
                        res_d.ap()[p0 + sub * 128:p0 + (sub + 1) * 128, :],
                        rs[:])

            # software pipeline: PE does q(p+1) while DVE/Act build Z(p)
            z0 = emit_q(0)
            z1 = emit_q(1)
            emit_v(0, z0)
            z2 = emit_q(2)
            emit_v(1, z1)
            z3 = emit_q(3)
            emit_v(2, z2)
            emit_v(3, z3)

    _legalize_waits(nc)
    return nc


def _programs(has_bias):
    key = ("ab", has_bias)
    if key not in _cache:
        _cache[key] = (_build_a(has_bias), _build_b(has_bias))
    return _cache[key]


def kernel(output, hidden_states, Wq, bq, Wk, bk, Wv, bv, gate, _trace=False):
    import ml_dtypes
    from concourse import bass_utils

    output = np.asarray(output, dtype=np.float32)
    hidden = np.asarray(hidden_states, dtype=np.float32)
    cst = _host_constants(
        np.asarray(Wq, np.float32), np.asarray(bq, np.float32),
        np.asarray(Wk, np.float32), np.asarray(bk, np.float32),
        np.asarray(Wv, np.float32), np.asarray(bv, np.float32),
        np.asarray(gate, np.float32))
    has_bias = cst["has_bias"]
    nca, ncb = _programs(has_bias)

    ac = np.ascontiguousarray
    chunks = [(c // 4, c % 4) for c in range(NCORES)]

    def ht_pack(b, j):
        hT = hidden[b, j * CHUNK:(j + 1) * CHUNK, :].T  # [1024, 2048]
        return ac(hT.reshape(8, 128, CHUNK).transpose(1, 0, 2)
                  .reshape(128, 8 * CHUNK)).astype(ml_dtypes.bfloat16)

    sharedA = {"MkP": cst["MkP"], "MvP": cst["MvP"]}
    if has_bias:
        sharedA["biask"] = cst["bk"]
        sharedA["biasv"] = cst["bv"]

    hts = [ht_pack(b, j) for (b, j) in chunks]
    in_a = []
    for c, (b, j) in enumerate(chunks):
        im = dict(sharedA)
        im["ht"] = hts[c]
        in_a.append(im)
    res_a = bass_utils.run_bass_kernel_spmd(
        nca, in_a, core_ids=list(range(NCORES)), trace=_trace)

    # host: causal prefix over per-chunk totals (fp32)
    cars = [np.asarray(res_a.results[c]["car"], np.float32)
            for c in range(NCORES)]
    inits = []
    for c, (b, j) in enumerate(chunks):
        p = np.zeros((128, 8), np.float32)
        for c2, (b2, j2) in enumerate(chunks):
            if b2 == b and j2 < j:
                p += cars[c2]
        inits.append(p)

    sharedB = {"MqP": cst["MqP"], "RP": cst["RP"]}
    if has_bias:
        sharedB["biasq"] = cst["bq"]

    in_b = []
    for c, (b, j) in enumerate(chunks):
        im = dict(sharedB)
        im["ht"] = hts[c]
        im["mem"] = res_a.results[c]["mem"]
        im["init"] = inits[c]
        im["outp"] = ac(output[b, j * CHUNK:(j + 1) * CHUNK, :])
        in_b.append(im)
    res_b = bass_utils.run_bass_kernel_spmd(
        ncb, in_b, core_ids=list(range(NCORES)), trace=_trace)

    out = np.empty((B, S, D), dtype=np.float32)
    for c, (b, j) in enumerate(chunks):
        out[b, j * CHUNK:(j + 1) * CHUNK, :] = res_b.results[c]["res"]
    if _trace:
        kernel._last = (res_a, res_b)
    return out


# revision 13
# speedup vs baseline: 1.5631x; 1.0903x over previous
"""Trainium2 Bass kernel for nn_HRRAdaptedAttention (B=2, S=8192, D=1024).

out = output + gate * irfft(cumsum_s(rfft(k)*rfft(v)) * conj(rfft(q))),
q/k/v = hidden @ W.T + b.

Sharding: (batch, seq) -> 8 chunks of 2048 positions, one per core.
The rfft/irfft are folded into the projection weights on the host, so on
device everything is bf16 matmuls, elementwise complex products, and a
per-frequency fp32-state scan over the sequence axis.

Packed spectrum (1024 rows, no separate nyquist matmuls):
  rows 0..511    = C-block: Re coefficients for f = 0..511
  rows 512..1023 = S-block: row 512 holds the nyquist (f=512, real)
                   channel in the otherwise-zero S_0 slot; rows 513.. are
                   Im for f = 1..511.
Partition 0 of each S-tile therefore carries f=512, which needs a few
single-partition fixups per panel (see comments at the fixup sites).

Launch A (per core): fk, fv (bf16 matmuls from host-transposed h^T);
the complex product's combine step is fused into the cumsum scan
(state = (u1 + state) - u2), mem (bf16) to DRAM; chunk totals are the
scan carries.  Host: exclusive prefix over chunk totals.
Launch B: fq; Z = (mem + carry) * fq with the carry folded in on the
Act engine (Identity + per-partition bias); values = Z @ R (gate/irfft
folded into R); res = output + values.
"""

import numpy as np

B, S, D = 2, 8192, 1024
NCORES = 8
CHUNK = 2048
PANEL = 512
NPANEL = CHUNK // PANEL
NDP = 8                  # 128-row tiles along the contraction (d) axis
NFT = 8                  # 128-row tiles along the packed frequency axis

_cache = {}
WARMUP_A = 8
WARMUP_B = 8


def _host_constants(Wq, bq, Wk, bk, Wv, bv, gate):
    import ml_dtypes

    d = np.arange(D, dtype=np.float64)
    f = np.arange(D // 2 + 1, dtype=np.float64)
    ang = 2.0 * np.pi * np.outer(d, f) / D
    C = np.cos(ang)              # [D, 513]
    Sm = -np.sin(ang)

    def fold_pack(W, sign_s=1.0):
        Wt = W.T.astype(np.float64)
        FC = Wt @ C              # [D, 513] Re part
        FS = sign_s * (Wt @ Sm)  # [D, 513] Im part
        P = np.empty((D, D), dtype=np.float64)
        P[:, 0:512] = FC[:, 0:512]
        P[:, 512] = FC[:, 512]          # nyquist -> S-block slot 0
        P[:, 513:1024] = FS[:, 1:512]
        return P

    MkP = fold_pack(Wk)
    MvP = fold_pack(Wv)
    MqP = fold_pack(Wq, sign_s=-1.0)     # conj(fq) folded

    g = float(np.asarray(gate).reshape(-1)[0])
    w = np.full(D // 2 + 1, 2.0)
    w[0] = 1.0
    w[512] = 1.0
    scale = (w * g / D)[:, None]
    A = scale * C.T                      # [513, D] coeff for Z_re
    Bm = scale * Sm.T                    # [513, D] coeff for Z_im
    RP = np.empty((D, D), dtype=np.float64)
    RP[0:512] = A[0:512]
    RP[512] = A[512]                     # nyquist coeff in S-block slot 0
    RP[513:1024] = Bm[1:512]

    def bias_pack(bvec, sign_s=1.0):
        b64 = np.asarray(bvec, np.float64)
        BC = b64 @ C
        BS = sign_s * (b64 @ Sm)
        p = np.empty(D, np.float64)
        p[0:512] = BC[0:512]
        p[512] = BC[512]
        p[513:1024] = BS[1:512]
        return p

    bkP = bias_pack(bk)
    bvP = bias_pack(bv)
    bqP = bias_pack(bq, sign_s=-1.0)

    def tile8(M):
        # [128p, 8192] with block i at cols i*1024..(i+1)*1024, from [1024, 1024]
        return np.ascontiguousarray(
            M.reshape(8, 128, 1024).transpose(1, 0, 2).reshape(128, 8192)
        ).astype(ml_dtypes.bfloat16)

    def col8(v):
        # [1024] -> [128, 8] with row block i in col i
        return np.ascontiguousarray(
            v.reshape(8, 128).T).astype(np.float32)

    return dict(MkP=tile8(MkP), MvP=tile8(MvP), MqP=tile8(MqP), RP=tile8(RP),
                bk=col8(bkP), bv=col8(bvP), bq=col8(bqP),
                has_bias=bool(np.any(bkP) or np.any(bvP) or np.any(bqP)))


_WAIT_EXEMPT = {
    "InstNoOp", "InstEventSemaphore", "InstUnconditionalBranch",
    "InstRegisterMove", "InstCall", "InstISA",
}


def _legalize_waits(nc, max_waits=1):
    """TRN2 instruction structs hold one sync-wait command; move extra waits
    onto same-engine nops inserted just before the instruction."""
    import bass_rust
    import concourse.mybir as mybir
    ctr = 0
    for fn in nc.m.functions:
        for blk in fn.blocks:
            new = []
            for inst in blk.instructions:
                if (type(inst).__name__ not in _WAIT_EXEMPT
                        and inst.sync_info is not None):
                    waits = list(inst.sync_info.on_wait)
                    if len(waits) > max_waits:
                        for w in waits[:-max_waits]:
                            nop = mybir.InstNoOp(
                                name=f"I-lglnop-{ctr}", ins=[], outs=[])
                            ctr += 1
                            nop.engine = inst.engine
                            nop.sync_info = bass_rust.SyncInfo(
                                on_wait=[w], on_update=[])
                            new.append(nop)
                        inst.sync_info = bass_rust.SyncInfo(
                            on_wait=waits[-max_waits:],
                            on_update=inst.sync_info.on_update)
                new.append(inst)
            blk.instructions = new


def _build_a(has_bias):
    import concourse.bass as bass
    import concourse.mybir as mybir
    import concourse.tile as tile
    F32, BF16 = mybir.dt.float32, mybir.dt.bfloat16
    AT = mybir.AluOpType
    AF = mybir.ActivationFunctionType

    nc = bass.Bass("TRN2", target_bir_lowering=False, debug=False,
                   num_devices=NCORES)
    ht_d = nc.dram_tensor("ht", [128, NDP * CHUNK], BF16, kind="ExternalInput")
    mk_d = nc.dram_tensor("MkP", [128, NDP * 1024], BF16, kind="ExternalInput")
    mv_d = nc.dram_tensor("MvP", [128, NDP * 1024], BF16, kind="ExternalInput")
    if has_bias:
        biask_d = nc.dram_tensor("biask", [128, 8], F32, kind="ExternalInput")
        biasv_d = nc.dram_tensor("biasv", [128, 8], F32, kind="ExternalInput")
    mem_d = nc.dram_tensor("mem", [NPANEL * 128, NFT * PANEL], BF16,
                           kind="ExternalOutput")
    car_d = nc.dram_tensor("car", [128, 8], F32, kind="ExternalOutput")

    with tile.TileContext(nc) as tc:
        with (
            tc.tile_pool(name="const", bufs=1) as cp,
            tc.tile_pool(name="wpool", bufs=1) as wp,
            tc.tile_pool(name="fkv", bufs=2) as fkp,
            tc.tile_pool(name="mem", bufs=2) as memp,
            tc.tile_pool(name="work", bufs=3) as wkp,
            tc.tile_pool(name="carp", bufs=1) as carp,
            tc.tile_pool(name="ps", bufs=8, space="PSUM") as psp,
        ):
            mk = wp.tile([128, NDP * 1024], BF16, tag="mk")
            mv = wp.tile([128, NDP * 1024], BF16, tag="mv")
            ht = wp.tile([128, NDP * CHUNK], BF16, tag="ht")
            # stream weights per-dp and ht per (dp, panel) so panel-0
            # matmuls only wait on ~5MB
            for dp in range(NDP):
                nc.sync.dma_start(mk[:, dp * 1024:(dp + 1) * 1024],
                                  mk_d.ap()[:, dp * 1024:(dp + 1) * 1024])
                nc.sync.dma_start(
                    ht[:, dp * CHUNK:dp * CHUNK + PANEL],
                    ht_d.ap()[:, dp * CHUNK:dp * CHUNK + PANEL])
                nc.sync.dma_start(mv[:, dp * 1024:(dp + 1) * 1024],
                                  mv_d.ap()[:, dp * 1024:(dp + 1) * 1024])
            for pp in range(1, NPANEL):
                for dp in range(NDP):
                    c0 = dp * CHUNK + pp * PANEL
                    nc.sync.dma_start(ht[:, c0:c0 + PANEL],
                                      ht_d.ap()[:, c0:c0 + PANEL])
            if has_bias:
                bk = cp.tile([128, 8], F32, tag="bk")
                nc.sync.dma_start(bk[:], biask_d.ap())
                bv = cp.tile([128, 8], F32, tag="bv")
                nc.sync.dma_start(bv[:], biasv_d.ap())
            car = carp.tile([128, 8], F32, tag="car")

            # PE warmup: keep the array busy during the initial DMA wait so
            # real matmuls start at full clock (p-state ramps after 3us of
            # continuous execution)
            wrm = cp.tile([128, PANEL], BF16, tag="wrm")
            nc.vector.memset(wrm[:], 0.0)
            wps = psp.tile([128, PANEL], F32, tag="ps", name="ps_warm")
            for _ in range(WARMUP_A):
                nc.tensor.matmul(wps[:], wrm[:, 0:128], wrm[:],
                                 start=True, stop=True)

            segs = [(0, PANEL), (PANEL, PANEL), (2 * PANEL, PANEL),
                    (3 * PANEL, PANEL // 2), (3 * PANEL + PANEL // 2,
                                              PANEL // 2)]
            for p, (p0, W) in enumerate(segs):
                rb = p0 // PANEL          # mem DRAM row block
                co = p0 % PANEL           # column offset within the block
                mem = memp.tile([128, NFT * PANEL], BF16, tag="mem",
                                name=f"mem_{p}")
                # two double-pair groups per panel; each uses all 8 PSUM
                # banks with dp-outer accumulation so panel 0 overlaps the
                # weight/ht streaming
                for g in range(2):
                    pr0 = g * 2
                    tiles = []
                    for i in (pr0, pr0 + 1):
                        for wnm, ft in (("k", i), ("k", 4 + i),
                                        ("v", i), ("v", 4 + i)):
                            tiles.append((wnm, ft))
                    ps = {key: psp.tile([128, PANEL], F32, tag="ps",
                                        name=f"ps_{p}_{g}_{key[0]}_{key[1]}")
                          for key in tiles}
                    fkv = {}

                    def copy_tile(key):
                        wnm, ft = key
                        t = fkp.tile([128, PANEL], BF16,
                                     tag=f"f_{wnm}_{ft % 2}_{ft // 4}",
                                     name=f"f_{p}_{wnm}_{ft}")
                        if has_bias:
                            bt = bk if wnm == "k" else bv
                            nc.scalar.activation(
                                t[:, :W], ps[key][:, :W], AF.Identity,
                                bias=bt[:, ft:ft + 1], scale=1.0)
                        else:
                            nc.scalar.copy(t[:, :W], ps[key][:, :W])
                        fkv[key] = t

                    def mm(key, dp):
                        wnm, ft = key
                        wt = mk if wnm == "k" else mv
                        nc.tensor.matmul(
                            ps[key][:, :W],
                            wt[:, dp * 1024 + ft * 128:
                               dp * 1024 + (ft + 1) * 128],
                            ht[:, dp * CHUNK + p0:dp * CHUNK + p0 + W],
                            start=(dp == 0), stop=(dp == NDP - 1))

                    if p == 0:
                        # dp-outer: overlaps the weight/ht streaming
                        for dp in range(NDP):
                            for key in tiles:
                                mm(key, dp)
                        for key in tiles:
                            copy_tile(key)
                    else:
                        # per-tile: Act copies/DVE chain pipeline behind PE
                        for key in tiles:
                            for dp in range(NDP):
                                mm(key, dp)
                            copy_tile(key)

                    for i in (pr0, pr0 + 1):
                        u1 = wkp.tile([128, PANEL], BF16, tag="u1")
                        u2 = wkp.tile([128, PANEL], BF16, tag="u2")
                        u3 = wkp.tile([128, PANEL], BF16, tag="u3")
                        u4 = wkp.tile([128, PANEL], BF16, tag="u4")
                        nc.vector.tensor_tensor(
                            u1[:, :W], fkv[("k", i)][:, :W],
                            fkv[("v", i)][:, :W], op=AT.mult)
                        nc.vector.tensor_tensor(
                            u2[:, :W], fkv[("k", 4 + i)][:, :W],
                            fkv[("v", 4 + i)][:, :W], op=AT.mult)
                        nc.vector.tensor_tensor(
                            u3[:, :W], fkv[("k", i)][:, :W],
                            fkv[("v", 4 + i)][:, :W], op=AT.mult)
                        nc.vector.tensor_tensor(
                            u4[:, :W], fkv[("k", 4 + i)][:, :W],
                            fkv[("v", i)][:, :W], op=AT.mult)
                        ci = slice(i * PANEL, i * PANEL + W)
                        si = slice((4 + i) * PANEL, (4 + i) * PANEL + W)
                        # combine fused into the scan: state=(u1+state)-u2
                        nc.vector.tensor_tensor_scan(
                            mem[:, ci], u1[:, :W], u2[:, :W],
                            0.0 if p == 0 else car[:, i:i + 1],
                            op0=AT.add, op1=AT.subtract)
                        nc.vector.tensor_tensor_scan(
                            mem[:, si], u3[:, :W], u4[:, :W],
                            0.0 if p == 0 else car[:, 4 + i:5 + i],
                            op0=AT.add, op1=AT.add)
                        if i == 0:
                            # partition 0 of the S-block is the nyquist
                            # channel: row 0 of the C-scan must not have
                            # subtracted u2 (true S_0 contribution is zero)
                            # and row 0 of the S-scan must be the cumsum of
                            # the nyquist product u2 itself.
                            ny = wkp.tile([1, PANEL], F32, tag="ny")
                            nc.vector.tensor_tensor_scan(
                                ny[:, :W], u2[0:1, :W], u2[0:1, :W], 0.0,
                                op0=AT.add, op1=AT.bypass)
                            nc.vector.tensor_tensor(
                                mem[0:1, ci], mem[0:1, ci], ny[:, :W],
                                op=AT.add)
                            nc.vector.tensor_scalar(
                                mem[0:1, si], ny[:, :W],
                                0.0 if p == 0 else car[0:1, 4:5], None,
                                op0=AT.add)
                        # chunk-total carries = scan state at segment end
                        nc.vector.tensor_copy(car[:, i:i + 1],
                                              mem[:, i * PANEL + W - 1:
                                                  i * PANEL + W])
                        nc.vector.tensor_copy(car[:, 4 + i:5 + i],
                                              mem[:, (4 + i) * PANEL + W - 1:
                                                  (4 + i) * PANEL + W])

                # one strided DMA for the whole segment's mem slices
                nc.sync.dma_start(
                    mem_d.ap()[rb * 128:(rb + 1) * 128, :]
                    .rearrange("p (ft w) -> p ft w", ft=NFT)[:, :, co:co + W],
                    mem[:].rearrange("p (ft w) -> p ft w", ft=NFT)[:, :, 0:W])

            nc.sync.dma_start(car_d.ap(), car[:])

    _legalize_waits(nc)
    return nc


def _build_b(has_bias):
    import concourse.bass as bass
    import concourse.mybir as mybir
    import concourse.tile as tile
    F32, BF16 = mybir.dt.float32, mybir.dt.bfloat16
    AT = mybir.AluOpType
    AF = mybir.ActivationFunctionType

    nc = bass.Bass("TRN2", target_bir_lowering=False, debug=False,
                   num_devices=NCORES)
    ht_d = nc.dram_tensor("ht", [128, NDP * CHUNK], BF16, kind="ExternalInput")
    mq_d = nc.dram_tensor("MqP", [128, NDP * 1024], BF16, kind="ExternalInput")
    r_d = nc.dram_tensor("RP", [128, NFT * 1024], BF16, kind="ExternalInput")
    mem_d = nc.dram_tensor("mem", [NPANEL * 128, NFT * PANEL], BF16,
                           kind="ExternalInput")
    init_d = nc.dram_tensor("init", [128, 8], F32, kind="ExternalInput")
    outp_d = nc.dram_tensor("outp", [CHUNK, D], F32, kind="ExternalInput")
    if has_bias:
        biasq_d = nc.dram_tensor("biasq", [128, 8], F32, kind="ExternalInput")
    res_d = nc.dram_tensor("res", [CHUNK, D], F32, kind="ExternalOutput")

    with tile.TileContext(nc) as tc:
        with (
            tc.tile_pool(name="const", bufs=1) as cp,
            tc.tile_pool(name="wpool", bufs=1) as wp,
            tc.tile_pool(name="qf", bufs=2) as qfp,
            tc.tile_pool(name="mp", bufs=2) as mpp,
            tc.tile_pool(name="z", bufs=2) as zp,
            tc.tile_pool(name="mem", bufs=2) as memp,
            tc.tile_pool(name="work", bufs=3) as wkp,
            tc.tile_pool(name="io", bufs=6) as iop,
            tc.tile_pool(name="rs", bufs=4) as rsp,
            tc.tile_pool(name="ps", bufs=8, space="PSUM") as psp,
        ):
            mq = wp.tile([128, NDP * 1024], BF16, tag="mq")
            ht = wp.tile([128, NDP * CHUNK], BF16, tag="ht")
            rp = wp.tile([128, NFT * 1024], BF16, tag="rp")
            for dp in range(NDP):
                nc.sync.dma_start(mq[:, dp * 1024:(dp + 1) * 1024],
                                  mq_d.ap()[:, dp * 1024:(dp + 1) * 1024])
                nc.sync.dma_start(
                    ht[:, dp * CHUNK:dp * CHUNK + PANEL],
                    ht_d.ap()[:, dp * CHUNK:dp * CHUNK + PANEL])
            car = cp.tile([128, 8], F32, tag="car")
            nc.sync.dma_start(car[:], init_d.ap())
            wrm = cp.tile([128, PANEL], BF16, tag="wrm")
            nc.vector.memset(wrm[:], 0.0)
            wps = psp.tile([128, PANEL], F32, tag="ps", name="ps_warm")
            for _ in range(WARMUP_B):
                nc.tensor.matmul(wps[:], wrm[:, 0:128], wrm[:],
                                 start=True, stop=True)
            mems = []
            for p in range(NPANEL):
                m = memp.tile([128, NFT * PANEL], BF16, tag="mem",
                              name=f"mem_{p}")
                mems.append(m)
            nc.sync.dma_start(mems[0][:], mem_d.ap()[0:128, :])
            for dp in range(NDP):
                c0 = dp * CHUNK + PANEL
                nc.sync.dma_start(ht[:, c0:c0 + PANEL],
                                  ht_d.ap()[:, c0:c0 + PANEL])
            for rt in range(NFT):
                nc.sync.dma_start(rp[:, rt * 1024:(rt + 1) * 1024],
                                  r_d.ap()[:, rt * 1024:(rt + 1) * 1024])
            for pp in range(2, NPANEL):
                for dp in range(NDP):
                    c0 = dp * CHUNK + pp * PANEL
                    nc.sync.dma_start(ht[:, c0:c0 + PANEL],
                                      ht_d.ap()[:, c0:c0 + PANEL])
            if has_bias:
                bq = cp.tile([128, 8], F32, tag="bq")
                nc.sync.dma_start(bq[:], biasq_d.ap())

            obs = {}

            def emit_q(p):
                p0 = p * PANEL
                qf = qfp.tile([128, NFT * PANEL], BF16, tag="qf",
                              name=f"qf_{p}")
                groups = [range(8)] if p == 0 else [range(4), range(4, 8)]
                for grp in groups:
                    ps = {ft: psp.tile([128, PANEL], F32, tag="ps",
                                       name=f"psq_{p}_{ft}")
                          for ft in grp}
                    for dp in range(NDP):
                        for ft in grp:
                            nc.tensor.matmul(
                                ps[ft][:],
                                mq[:, dp * 1024 + ft * 128:
                                   dp * 1024 + (ft + 1) * 128],
                                ht[:, dp * CHUNK + p0:dp * CHUNK + p0 + PANEL],
                                start=(dp == 0), stop=(dp == NDP - 1))
                    for ft in grp:
                        sl = qf[:, ft * PANEL:(ft + 1) * PANEL]
                        if has_bias:
                            nc.scalar.activation(
                                sl, ps[ft][:], AF.Identity,
                                bias=bq[:, ft:ft + 1], scale=1.0)
                        else:
                            nc.scalar.copy(sl, ps[ft][:])
                # prefetch next panel's mem and this panel's output rows
                if p + 1 < NPANEL:
                    nc.sync.dma_start(mems[p + 1][:],
                                      mem_d.ap()[(p + 1) * 128:(p + 2) * 128, :])
                obl = []
                for sub in range(PANEL // 128):
                    ob = iop.tile([128, D], F32, tag="ob",
                                  name=f"ob_{p}_{sub}")
                    nc.sync.dma_start(
                        ob[:],
                        outp_d.ap()[p0 + sub * 128:p0 + (sub + 1) * 128, :])
                    obl.append(ob)
                obs[p] = obl

                # memP = mem + carry on the Act engine (Identity + bias)
                mem = mems[p]
                mp = mpp.tile([128, NFT * PANEL], BF16, tag="mp",
                              name=f"mp_{p}")
                for ft in range(NFT):
                    sl = slice(ft * PANEL, (ft + 1) * PANEL)
                    nc.scalar.activation(mp[:, sl], mem[:, sl], AF.Identity,
                                         bias=car[:, ft:ft + 1], scale=1.0)

                zc = zp.tile([128, 4 * PANEL], BF16, tag="zc", name=f"zc_{p}")
                zs = zp.tile([128, 4 * PANEL], BF16, tag="zs", name=f"zs_{p}")
                for i in range(4):
                    ci = slice(i * PANEL, (i + 1) * PANEL)
                    si = slice((4 + i) * PANEL, (5 + i) * PANEL)
                    u1 = wkp.tile([128, PANEL], BF16, tag="u1")
                    u2 = wkp.tile([128, PANEL], BF16, tag="u2")
                    u3 = wkp.tile([128, PANEL], BF16, tag="u3")
                    u4 = wkp.tile([128, PANEL], BF16, tag="u4")
                    nc.vector.tensor_tensor(u1[:], mp[:, ci], qf[:, ci],
                                            op=AT.mult)
                    nc.vector.tensor_tensor(u2[:], mp[:, si], qf[:, si],
                                            op=AT.mult)
                    nc.vector.tensor_tensor(u3[:], mp[:, ci], qf[:, si],
                                            op=AT.mult)
                    nc.vector.tensor_tensor(u4[:], mp[:, si], qf[:, ci],
                                            op=AT.mult)
                    nc.vector.tensor_tensor(zc[:, ci], u1[:], u2[:],
                                            op=AT.subtract)
                    nc.vector.tensor_tensor(zs[:, ci], u3[:], u4[:],
                                            op=AT.add)
                    if i == 0:
                        # S-block partition 0 is the nyquist channel: Z_re
                        # row0 is the plain DC product u1 and the S-slot
                        # row0 carries Z_512 = u2 (R row 512 holds A512).
                        nc.vector.tensor_copy(zc[0:1, ci], u1[0:1, :])
                        nc.vector.tensor_copy(zs[0:1, ci], u2[0:1, :])
                return zc, zs

            def emit_v(p, zcs):
                p0 = p * PANEL
                zc, zs = zcs
                for sub in range(PANEL // 128):
                    ob = obs[p][sub]
                    rs = rsp.tile([128, D], F32, tag="rs")
                    s0, s1 = sub * 128, (sub + 1) * 128
                    for half in range(2):
                        pv = psp.tile([128, 512], F32, tag="ps",
                                      name=f"pv_{p}_{sub}_{half}")
                        d0 = half * 512
                        for i in range(4):
                            nc.tensor.matmul(
                                pv[:], zc[:, i * PANEL + s0:i * PANEL + s1],
                                rp[:, i * 1024 + d0:i * 1024 + d0 + 512],
                                start=(i == 0), stop=False)
                        for i in range(4):
                            nc.tensor.matmul(
                                pv[:], zs[:, i * PANEL + s0:i * PANEL + s1],
                                rp[:, (4 + i) * 1024 + d0:
                                   (4 + i) * 1024 + d0 + 512],
                                start=False, stop=(i == 3))
                        pvc = wkp.tile([128, 512], F32, tag="pvc")
                        nc.scalar.copy(pvc[:], pv[:])
                        nc.vector.tensor_tensor(rs[:, d0:d0 + 512], pvc[:],
                                                ob[:, d0:d0 + 512], op=AT.add)
                        nc.sync.dma_start(
                            res_d.ap()[p0 + sub * 128:p0 + (sub + 1) * 128,
                                       d0:d0 + 512],
                            rs[:, d0:d0 + 512])

            # software pipeline: PE does q(p+1) while DVE/Act build Z(p)
            z0 = emit_q(0)
            z1 = emit_q(1)
            emit_v(0, z0)
            z2 = emit_q(2)
            emit_v(1, z1)
            z3 = emit_q(3)
            emit_v(2, z2)
            emit_v(3, z3)

    _legalize_waits(nc)
    return nc


def _programs(has_bias):
    key = ("ab", has_bias)
    if key not in _cache:
        _cache[key] = (_build_a(has_bias), _build_b(has_bias))
    return _cache[key]


def kernel(output, hidden_states, Wq, bq, Wk, bk, Wv, bv, gate, _trace=False):
    import ml_dtypes
    from concourse import bass_utils

    output = np.asarray(output, dtype=np.float32)
    hidden = np.asarray(hidden_states, dtype=np.float32)
    cst = _host_constants(
        np.asarray(Wq, np.float32), np.asarray(bq, np.float32),
        np.asarray(Wk, np.float32), np.asarray(bk, np.float32),
        np.asarray(Wv, np.float32), np.asarray(bv, np.float32),
        np.asarray(gate, np.float32))
    has_bias = cst["has_bias"]
    nca, ncb = _programs(has_bias)

    ac = np.ascontiguousarray
    chunks = [(c // 4, c % 4) for c in range(NCORES)]

    def ht_pack(b, j):
        hT = hidden[b, j * CHUNK:(j + 1) * CHUNK, :].T  # [1024, 2048]
        return ac(hT.reshape(8, 128, CHUNK).transpose(1, 0, 2)
                  .reshape(128, 8 * CHUNK)).astype(ml_dtypes.bfloat16)

    sharedA = {"MkP": cst["MkP"], "MvP": cst["MvP"]}
    if has_bias:
        sharedA["biask"] = cst["bk"]
        sharedA["biasv"] = cst["bv"]

    hts = [ht_pack(b, j) for (b, j) in chunks]
    in_a = []
    for c, (b, j) in enumerate(chunks):
        im = dict(sharedA)
        im["ht"] = hts[c]
        in_a.append(im)
    res_a = bass_utils.run_bass_kernel_spmd(
        nca, in_a, core_ids=list(range(NCORES)), trace=_trace)

    # host: causal prefix over per-chunk totals (fp32)
    cars = [np.asarray(res_a.results[c]["car"], np.float32)
            for c in range(NCORES)]
    inits = []
    for c, (b, j) in enumerate(chunks):
        p = np.zeros((128, 8), np.float32)
        for c2, (b2, j2) in enumerate(chunks):
            if b2 == b and j2 < j:
                p += cars[c2]
        inits.append(p)

    sharedB = {"MqP": cst["MqP"], "RP": cst["RP"]}
    if has_bias:
        sharedB["biasq"] = cst["bq"]

    in_b = []
    for c, (b, j) in enumerate(chunks):
        im = dict(sharedB)
        im["ht"] = hts[c]
        im["mem"] = res_a.results[c]["mem"]
        im["init"] = inits[c]
        im["outp"] = ac(output[b, j * CHUNK:(j + 1) * CHUNK, :])
        in_b.append(im)
    res_b = bass_utils.run_bass_kernel_spmd(
        ncb, in_b, core_ids=list(range(NCORES)), trace=_trace)

    out = np.empty((B, S, D), dtype=np.float32)
    for c, (b, j) in enumerate(chunks):
        out[b, j * CHUNK:(j + 1) * CHUNK, :] = res_b.results[c]["res"]
    if _trace:
        kernel._last = (res_a, res_b)
    return out


# revision 17
# speedup vs baseline: 1.5787x; 1.0100x over previous
"""Trainium2 Bass kernel for nn_HRRAdaptedAttention (B=2, S=8192, D=1024).

out = output + gate * irfft(cumsum_s(rfft(k)*rfft(v)) * conj(rfft(q))),
q/k/v = hidden @ W.T + b.

Sharding: (batch, seq) -> 8 chunks of 2048 positions, one per core.
The rfft/irfft are folded into the projection weights on the host, so on
device everything is bf16 matmuls, elementwise complex products, and a
per-frequency fp32-state scan over the sequence axis.

Packed spectrum (1024 rows, no separate nyquist matmuls):
  rows 0..511    = C-block: Re coefficients for f = 0..511
  rows 512..1023 = S-block: row 512 holds the nyquist (f=512, real)
                   channel in the otherwise-zero S_0 slot; rows 513.. are
                   Im for f = 1..511.
Partition 0 of each S-tile therefore carries f=512, which needs a few
single-partition fixups per panel (see comments at the fixup sites).

Launch A (per core): fk, fv (bf16 matmuls from host-transposed h^T);
the complex product's combine step is fused into the cumsum scan
(state = (u1 + state) - u2), mem (bf16) to DRAM; chunk totals are the
scan carries.  Host: exclusive prefix over chunk totals.
Launch B: fq; Z = (mem + carry) * fq with the carry folded in on the
Act engine (Identity + per-partition bias); values = Z @ R (gate/irfft
folded into R); res = output + values.
"""

import numpy as np

B, S, D = 2, 8192, 1024
NCORES = 8
CHUNK = 2048
PANEL = 512
NPANEL = CHUNK // PANEL
NDP = 8                  # 128-row tiles along the contraction (d) axis
NFT = 8                  # 128-row tiles along the packed frequency axis

_cache = {}
WARMUP_A = 24
WARMUP_B = 24


def _host_constants(Wq, bq, Wk, bk, Wv, bv, gate):
    import ml_dtypes

    d = np.arange(D, dtype=np.float64)
    f = np.arange(D // 2 + 1, dtype=np.float64)
    ang = 2.0 * np.pi * np.outer(d, f) / D
    C = np.cos(ang)              # [D, 513]
    Sm = -np.sin(ang)

    def fold_pack(W, sign_s=1.0):
        Wt = W.T.astype(np.float64)
        FC = Wt @ C              # [D, 513] Re part
        FS = sign_s * (Wt @ Sm)  # [D, 513] Im part
        P = np.empty((D, D), dtype=np.float64)
        P[:, 0:512] = FC[:, 0:512]
        P[:, 512] = FC[:, 512]          # nyquist -> S-block slot 0
        P[:, 513:1024] = FS[:, 1:512]
        return P

    MkP = fold_pack(Wk)
    MvP = fold_pack(Wv)
    MqP = fold_pack(Wq, sign_s=-1.0)     # conj(fq) folded

    g = float(np.asarray(gate).reshape(-1)[0])
    w = np.full(D // 2 + 1, 2.0)
    w[0] = 1.0
    w[512] = 1.0
    scale = (w * g / D)[:, None]
    A = scale * C.T                      # [513, D] coeff for Z_re
    Bm = scale * Sm.T                    # [513, D] coeff for Z_im
    RP = np.empty((D, D), dtype=np.float64)
    RP[0:512] = A[0:512]
    RP[512] = A[512]                     # nyquist coeff in S-block slot 0
    RP[513:1024] = Bm[1:512]

    def bias_pack(bvec, sign_s=1.0):
        b64 = np.asarray(bvec, np.float64)
        BC = b64 @ C
        BS = sign_s * (b64 @ Sm)
        p = np.empty(D, np.float64)
        p[0:512] = BC[0:512]
        p[512] = BC[512]
        p[513:1024] = BS[1:512]
        return p

    bkP = bias_pack(bk)
    bvP = bias_pack(bv)
    bqP = bias_pack(bq, sign_s=-1.0)

    def tile8(M):
        # [128p, 8192] with block i at cols i*1024..(i+1)*1024, from [1024, 1024]
        return np.ascontiguousarray(
            M.reshape(8, 128, 1024).transpose(1, 0, 2).reshape(128, 8192)
        ).astype(ml_dtypes.bfloat16)

    def col8(v):
        # [1024] -> [128, 8] with row block i in col i
        return np.ascontiguousarray(
            v.reshape(8, 128).T).astype(np.float32)

    return dict(MkP=tile8(MkP), MvP=tile8(MvP), MqP=tile8(MqP), RP=tile8(RP),
                bk=col8(bkP), bv=col8(bvP), bq=col8(bqP),
                has_bias=bool(np.any(bkP) or np.any(bvP) or np.any(bqP)))


_WAIT_EXEMPT = {
    "InstNoOp", "InstEventSemaphore", "InstUnconditionalBranch",
    "InstRegisterMove", "InstCall", "InstISA",
}


def _legalize_waits(nc, max_waits=1):
    """TRN2 instruction structs hold one sync-wait command; move extra waits
    onto same-engine nops inserted just before the instruction."""
    import bass_rust
    import concourse.mybir as mybir
    ctr = 0
    for fn in nc.m.functions:
        for blk in fn.blocks:
            new = []
            for inst in blk.instructions:
                if (type(inst).__name__ not in _WAIT_EXEMPT
                        and inst.sync_info is not None):
                    waits = list(inst.sync_info.on_wait)
                    if len(waits) > max_waits:
                        for w in waits[:-max_waits]:
                            nop = mybir.InstNoOp(
                                name=f"I-lglnop-{ctr}", ins=[], outs=[])
                            ctr += 1
                            nop.engine = inst.engine
                            nop.sync_info = bass_rust.SyncInfo(
                                on_wait=[w], on_update=[])
                            new.append(nop)
                        inst.sync_info = bass_rust.SyncInfo(
                            on_wait=waits[-max_waits:],
                            on_update=inst.sync_info.on_update)
                new.append(inst)
            blk.instructions = new


def _build_a(has_bias):
    import concourse.bass as bass
    import concourse.mybir as mybir
    import concourse.tile as tile
    F32, BF16 = mybir.dt.float32, mybir.dt.bfloat16
    AT = mybir.AluOpType
    AF = mybir.ActivationFunctionType

    nc = bass.Bass("TRN2", target_bir_lowering=False, debug=False,
                   num_devices=NCORES)
    ht_d = nc.dram_tensor("ht", [128, NDP * CHUNK], BF16, kind="ExternalInput")
    mk_d = nc.dram_tensor("MkP", [128, NDP * 1024], BF16, kind="ExternalInput")
    mv_d = nc.dram_tensor("MvP", [128, NDP * 1024], BF16, kind="ExternalInput")
    if has_bias:
        biask_d = nc.dram_tensor("biask", [128, 8], F32, kind="ExternalInput")
        biasv_d = nc.dram_tensor("biasv", [128, 8], F32, kind="ExternalInput")
    mem_d = nc.dram_tensor("mem", [NPANEL * 128, NFT * PANEL], BF16,
                           kind="ExternalOutput")
    car_d = nc.dram_tensor("car", [128, 8], F32, kind="ExternalOutput")

    with tile.TileContext(nc) as tc:
        with (
            tc.tile_pool(name="const", bufs=1) as cp,
            tc.tile_pool(name="wpool", bufs=1) as wp,
            tc.tile_pool(name="fkv", bufs=2) as fkp,
            tc.tile_pool(name="mem", bufs=2) as memp,
            tc.tile_pool(name="work", bufs=3) as wkp,
            tc.tile_pool(name="carp", bufs=1) as carp,
            tc.tile_pool(name="ps", bufs=8, space="PSUM") as psp,
        ):
            mk = wp.tile([128, NDP * 1024], BF16, tag="mk")
            mv = wp.tile([128, NDP * 1024], BF16, tag="mv")
            ht = wp.tile([128, NDP * CHUNK], BF16, tag="ht")
            # stream weights per-dp and ht per (dp, panel) so panel-0
            # matmuls only wait on ~5MB
            for dp in range(NDP):
                nc.sync.dma_start(mk[:, dp * 1024:(dp + 1) * 1024],
                                  mk_d.ap()[:, dp * 1024:(dp + 1) * 1024])
                nc.sync.dma_start(
                    ht[:, dp * CHUNK:dp * CHUNK + PANEL],
                    ht_d.ap()[:, dp * CHUNK:dp * CHUNK + PANEL])
                nc.sync.dma_start(mv[:, dp * 1024:(dp + 1) * 1024],
                                  mv_d.ap()[:, dp * 1024:(dp + 1) * 1024])
            for pp in range(1, NPANEL):
                for dp in range(NDP):
                    c0 = dp * CHUNK + pp * PANEL
                    nc.sync.dma_start(ht[:, c0:c0 + PANEL],
                                      ht_d.ap()[:, c0:c0 + PANEL])
            if has_bias:
                bk = cp.tile([128, 8], F32, tag="bk")
                nc.sync.dma_start(bk[:], biask_d.ap())
                bv = cp.tile([128, 8], F32, tag="bv")
                nc.sync.dma_start(bv[:], biasv_d.ap())
            car = carp.tile([128, 8], F32, tag="car")

            # PE warmup: keep the array busy during the initial DMA wait so
            # real matmuls start at full clock (p-state ramps after 3us of
            # continuous execution)
            wrm = cp.tile([128, PANEL], BF16, tag="wrm")
            nc.vector.memset(wrm[:, 0:128], 0.0)
            wps = psp.tile([128, PANEL], F32, tag="ps", name="ps_warm")
            for _ in range(WARMUP_A):
                nc.tensor.matmul(wps[:, 0:128], wrm[:, 0:128],
                                 wrm[:, 0:128], start=True, stop=True)

            segs = [(0, PANEL), (PANEL, PANEL), (2 * PANEL, PANEL),
                    (3 * PANEL, 256), (3 * PANEL + 256, 128),
                    (3 * PANEL + 384, 128)]
            for p, (p0, W) in enumerate(segs):
                rb = p0 // PANEL          # mem DRAM row block
                co = p0 % PANEL           # column offset within the block
                mem = memp.tile([128, NFT * PANEL], BF16, tag="mem",
                                name=f"mem_{p}")
                # two double-pair groups per panel; each uses all 8 PSUM
                # banks with dp-outer accumulation so panel 0 overlaps the
                # weight/ht streaming
                for g in range(2):
                    pr0 = g * 2
                    tiles = []
                    for i in (pr0, pr0 + 1):
                        for wnm, ft in (("k", i), ("k", 4 + i),
                                        ("v", i), ("v", 4 + i)):
                            tiles.append((wnm, ft))
                    ps = {key: psp.tile([128, PANEL], F32, tag="ps",
                                        name=f"ps_{p}_{g}_{key[0]}_{key[1]}")
                          for key in tiles}
                    fkv = {}

                    def copy_tile(key):
                        wnm, ft = key
                        t = fkp.tile([128, PANEL], BF16,
                                     tag=f"f_{wnm}_{ft % 2}_{ft // 4}",
                                     name=f"f_{p}_{wnm}_{ft}")
                        if has_bias:
                            bt = bk if wnm == "k" else bv
                            nc.scalar.activation(
                                t[:, :W], ps[key][:, :W], AF.Identity,
                                bias=bt[:, ft:ft + 1], scale=1.0)
                        else:
                            nc.scalar.copy(t[:, :W], ps[key][:, :W])
                        fkv[key] = t

                    def mm(key, dp):
                        wnm, ft = key
                        wt = mk if wnm == "k" else mv
                        nc.tensor.matmul(
                            ps[key][:, :W],
                            wt[:, dp * 1024 + ft * 128:
                               dp * 1024 + (ft + 1) * 128],
                            ht[:, dp * CHUNK + p0:dp * CHUNK + p0 + W],
                            start=(dp == 0), stop=(dp == NDP - 1))

                    if p == 0:
                        # dp-outer: overlaps the weight/ht streaming
                        for dp in range(NDP):
                            for key in tiles:
                                mm(key, dp)
                        for key in tiles:
                            copy_tile(key)
                    else:
                        # per-tile: Act copies/DVE chain pipeline behind PE
                        for key in tiles:
                            for dp in range(NDP):
                                mm(key, dp)
                            copy_tile(key)

                    for i in (pr0, pr0 + 1):
                        u1 = wkp.tile([128, PANEL], BF16, tag="u1")
                        u2 = wkp.tile([128, PANEL], BF16, tag="u2")
                        u3 = wkp.tile([128, PANEL], BF16, tag="u3")
                        u4 = wkp.tile([128, PANEL], BF16, tag="u4")
                        nc.vector.tensor_tensor(
                            u1[:, :W], fkv[("k", i)][:, :W],
                            fkv[("v", i)][:, :W], op=AT.mult)
                        nc.vector.tensor_tensor(
                            u2[:, :W], fkv[("k", 4 + i)][:, :W],
                            fkv[("v", 4 + i)][:, :W], op=AT.mult)
                        nc.vector.tensor_tensor(
                            u3[:, :W], fkv[("k", i)][:, :W],
                            fkv[("v", 4 + i)][:, :W], op=AT.mult)
                        nc.vector.tensor_tensor(
                            u4[:, :W], fkv[("k", 4 + i)][:, :W],
                            fkv[("v", i)][:, :W], op=AT.mult)
                        ci = slice(i * PANEL, i * PANEL + W)
                        si = slice((4 + i) * PANEL, (4 + i) * PANEL + W)
                        # combine fused into the scan: state=(u1+state)-u2
                        nc.vector.tensor_tensor_scan(
                            mem[:, ci], u1[:, :W], u2[:, :W],
                            0.0 if p == 0 else car[:, i:i + 1],
                            op0=AT.add, op1=AT.subtract)
                        nc.vector.tensor_tensor_scan(
                            mem[:, si], u3[:, :W], u4[:, :W],
                            0.0 if p == 0 else car[:, 4 + i:5 + i],
                            op0=AT.add, op1=AT.add)
                        if i == 0:
                            # partition 0 of the S-block is the nyquist
                            # channel: row 0 of the C-scan must not have
                            # subtracted u2 (true S_0 contribution is zero)
                            # and row 0 of the S-scan must be the cumsum of
                            # the nyquist product u2 itself.
                            ny = wkp.tile([1, PANEL], F32, tag="ny")
                            nc.vector.tensor_tensor_scan(
                                ny[:, :W], u2[0:1, :W], u2[0:1, :W], 0.0,
                                op0=AT.add, op1=AT.bypass)
                            nc.vector.tensor_tensor(
                                mem[0:1, ci], mem[0:1, ci], ny[:, :W],
                                op=AT.add)
                            nc.vector.tensor_scalar(
                                mem[0:1, si], ny[:, :W],
                                0.0 if p == 0 else car[0:1, 4:5], None,
                                op0=AT.add)
                        # chunk-total carries = scan state at segment end
                        nc.vector.tensor_copy(car[:, i:i + 1],
                                              mem[:, i * PANEL + W - 1:
                                                  i * PANEL + W])
                        nc.vector.tensor_copy(car[:, 4 + i:5 + i],
                                              mem[:, (4 + i) * PANEL + W - 1:
                                                  (4 + i) * PANEL + W])

                # one strided DMA for the whole segment's mem slices
                nc.sync.dma_start(
                    mem_d.ap()[rb * 128:(rb + 1) * 128, :]
                    .rearrange("p (ft w) -> p ft w", ft=NFT)[:, :, co:co + W],
                    mem[:].rearrange("p (ft w) -> p ft w", ft=NFT)[:, :, 0:W])

            nc.sync.dma_start(car_d.ap(), car[:])

    _legalize_waits(nc)
    return nc


def _build_b(has_bias):
    import concourse.bass as bass
    import concourse.mybir as mybir
    import concourse.tile as tile
    F32, BF16 = mybir.dt.float32, mybir.dt.bfloat16
    AT = mybir.AluOpType
    AF = mybir.ActivationFunctionType

    nc = bass.Bass("TRN2", target_bir_lowering=False, debug=False,
                   num_devices=NCORES)
    ht_d = nc.dram_tensor("ht", [128, NDP * CHUNK], BF16, kind="ExternalInput")
    mq_d = nc.dram_tensor("MqP", [128, NDP * 1024], BF16, kind="ExternalInput")
    r_d = nc.dram_tensor("RP", [128, NFT * 1024], BF16, kind="ExternalInput")
    mem_d = nc.dram_tensor("mem", [NPANEL * 128, NFT * PANEL], BF16,
                           kind="ExternalInput")
    init_d = nc.dram_tensor("init", [128, 8], F32, kind="ExternalInput")
    outp_d = nc.dram_tensor("outp", [CHUNK, D], F32, kind="ExternalInput")
    if has_bias:
        biasq_d = nc.dram_tensor("biasq", [128, 8], F32, kind="ExternalInput")
    res_d = nc.dram_tensor("res", [CHUNK, D], F32, kind="ExternalOutput")

    with tile.TileContext(nc) as tc:
        with (
            tc.tile_pool(name="const", bufs=1) as cp,
            tc.tile_pool(name="wpool", bufs=1) as wp,
            tc.tile_pool(name="qf", bufs=2) as qfp,
            tc.tile_pool(name="mp", bufs=2) as mpp,
            tc.tile_pool(name="z", bufs=2) as zp,
            tc.tile_pool(name="mem", bufs=2) as memp,
            tc.tile_pool(name="work", bufs=3) as wkp,
            tc.tile_pool(name="io", bufs=6) as iop,
            tc.tile_pool(name="rs", bufs=4) as rsp,
            tc.tile_pool(name="ps", bufs=8, space="PSUM") as psp,
        ):
            mq = wp.tile([128, NDP * 1024], BF16, tag="mq")
            ht = wp.tile([128, NDP * CHUNK], BF16, tag="ht")
            rp = wp.tile([128, NFT * 1024], BF16, tag="rp")
            for dp in range(NDP):
                nc.sync.dma_start(mq[:, dp * 1024:(dp + 1) * 1024],
                                  mq_d.ap()[:, dp * 1024:(dp + 1) * 1024])
                nc.sync.dma_start(
                    ht[:, dp * CHUNK:dp * CHUNK + PANEL],
                    ht_d.ap()[:, dp * CHUNK:dp * CHUNK + PANEL])
            car = cp.tile([128, 8], F32, tag="car")
            nc.sync.dma_start(car[:], init_d.ap())
            wrm = cp.tile([128, PANEL], BF16, tag="wrm")
            nc.vector.memset(wrm[:, 0:128], 0.0)
            wps = psp.tile([128, PANEL], F32, tag="ps", name="ps_warm")
            for _ in range(WARMUP_B):
                nc.tensor.matmul(wps[:, 0:128], wrm[:, 0:128],
                                 wrm[:, 0:128], start=True, stop=True)
            mems = []
            for p in range(NPANEL):
                m = memp.tile([128, NFT * PANEL], BF16, tag="mem",
                              name=f"mem_{p}")
                mems.append(m)
            nc.sync.dma_start(mems[0][:], mem_d.ap()[0:128, :])
            for dp in range(NDP):
                c0 = dp * CHUNK + PANEL
                nc.sync.dma_start(ht[:, c0:c0 + PANEL],
                                  ht_d.ap()[:, c0:c0 + PANEL])
            for rt in range(NFT):
                nc.sync.dma_start(rp[:, rt * 1024:(rt + 1) * 1024],
                                  r_d.ap()[:, rt * 1024:(rt + 1) * 1024])
            for pp in range(2, NPANEL):
                for dp in range(NDP):
                    c0 = dp * CHUNK + pp * PANEL
                    nc.sync.dma_start(ht[:, c0:c0 + PANEL],
                                      ht_d.ap()[:, c0:c0 + PANEL])
            if has_bias:
                bq = cp.tile([128, 8], F32, tag="bq")
                nc.sync.dma_start(bq[:], biasq_d.ap())

            obs = {}

            def emit_q(p):
                p0 = p * PANEL
                qf = qfp.tile([128, NFT * PANEL], BF16, tag="qf",
                              name=f"qf_{p}")
                groups = [range(8)] if p == 0 else [range(4), range(4, 8)]
                for grp in groups:
                    ps = {ft: psp.tile([128, PANEL], F32, tag="ps",
                                       name=f"psq_{p}_{ft}")
                          for ft in grp}
                    for dp in range(NDP):
                        for ft in grp:
                            nc.tensor.matmul(
                                ps[ft][:],
                                mq[:, dp * 1024 + ft * 128:
                                   dp * 1024 + (ft + 1) * 128],
                                ht[:, dp * CHUNK + p0:dp * CHUNK + p0 + PANEL],
                                start=(dp == 0), stop=(dp == NDP - 1))
                    for ft in grp:
                        sl = qf[:, ft * PANEL:(ft + 1) * PANEL]
                        if has_bias:
                            nc.scalar.activation(
                                sl, ps[ft][:], AF.Identity,
                                bias=bq[:, ft:ft + 1], scale=1.0)
                        else:
                            nc.scalar.copy(sl, ps[ft][:])
                # prefetch next panel's mem and this panel's output rows
                if p + 1 < NPANEL:
                    nc.sync.dma_start(mems[p + 1][:],
                                      mem_d.ap()[(p + 1) * 128:(p + 2) * 128, :])
                obl = []
                for sub in range(PANEL // 128):
                    ob = iop.tile([128, D], F32, tag="ob",
                                  name=f"ob_{p}_{sub}")
                    nc.sync.dma_start(
                        ob[:],
                        outp_d.ap()[p0 + sub * 128:p0 + (sub + 1) * 128, :])
                    obl.append(ob)
                obs[p] = obl

                # memP = mem + carry on the Act engine (Identity + bias)
                mem = mems[p]
                mp = mpp.tile([128, NFT * PANEL], BF16, tag="mp",
                              name=f"mp_{p}")
                for ft in range(NFT):
                    sl = slice(ft * PANEL, (ft + 1) * PANEL)
                    nc.scalar.activation(mp[:, sl], mem[:, sl], AF.Identity,
                                         bias=car[:, ft:ft + 1], scale=1.0)

                zc = zp.tile([128, 4 * PANEL], BF16, tag="zc", name=f"zc_{p}")
                zs = zp.tile([128, 4 * PANEL], BF16, tag="zs", name=f"zs_{p}")
                for i in range(4):
                    ci = slice(i * PANEL, (i + 1) * PANEL)
                    si = slice((4 + i) * PANEL, (5 + i) * PANEL)
                    u1 = wkp.tile([128, PANEL], BF16, tag="u1")
                    u2 = wkp.tile([128, PANEL], BF16, tag="u2")
                    u3 = wkp.tile([128, PANEL], BF16, tag="u3")
                    u4 = wkp.tile([128, PANEL], BF16, tag="u4")
                    nc.vector.tensor_tensor(u1[:], mp[:, ci], qf[:, ci],
                                            op=AT.mult)
                    nc.vector.tensor_tensor(u2[:], mp[:, si], qf[:, si],
                                            op=AT.mult)
                    nc.vector.tensor_tensor(u3[:], mp[:, ci], qf[:, si],
                                            op=AT.mult)
                    nc.vector.tensor_tensor(u4[:], mp[:, si], qf[:, ci],
                                            op=AT.mult)
                    nc.vector.tensor_tensor(zc[:, ci], u1[:], u2[:],
                                            op=AT.subtract)
                    nc.vector.tensor_tensor(zs[:, ci], u3[:], u4[:],
                                            op=AT.add)
                    if i == 0:
                        # S-block partition 0 is the nyquist channel: Z_re
                        # row0 is the plain DC product u1 and the S-slot
                        # row0 carries Z_512 = u2 (R row 512 holds A512).
                        nc.vector.tensor_copy(zc[0:1, ci], u1[0:1, :])
                        nc.vector.tensor_copy(zs[0:1, ci], u2[0:1, :])
                return zc, zs

            def emit_v(p, zcs):
                p0 = p * PANEL
                zc, zs = zcs
                for sub in range(PANEL // 128):
                    ob = obs[p][sub]
                    rs = rsp.tile([128, D], F32, tag="rs")
                    s0, s1 = sub * 128, (sub + 1) * 128
                    last = (p == NPANEL - 1 and sub == PANEL // 128 - 1)
                    dws = 256 if last else 512
                    for half in range(1024 // dws):
                        pv = psp.tile([128, 512], F32, tag="ps",
                                      name=f"pv_{p}_{sub}_{half}")
                        d0 = half * dws
                        for i in range(4):
                            nc.tensor.matmul(
                                pv[:, :dws],
                                zc[:, i * PANEL + s0:i * PANEL + s1],
                                rp[:, i * 1024 + d0:i * 1024 + d0 + dws],
                                start=(i == 0), stop=False)
                        for i in range(4):
                            nc.tensor.matmul(
                                pv[:, :dws],
                                zs[:, i * PANEL + s0:i * PANEL + s1],
                                rp[:, (4 + i) * 1024 + d0:
                                   (4 + i) * 1024 + d0 + dws],
                                start=False, stop=(i == 3))
                        pvc = wkp.tile([128, 512], F32, tag="pvc")
                        nc.scalar.copy(pvc[:, :dws], pv[:, :dws])
                        nc.vector.tensor_tensor(rs[:, d0:d0 + dws],
                                                pvc[:, :dws],
                                                ob[:, d0:d0 + dws], op=AT.add)
                        nc.sync.dma_start(
                            res_d.ap()[p0 + sub * 128:p0 + (sub + 1) * 128,
                                       d0:d0 + dws],
                            rs[:, d0:d0 + dws])

            # software pipeline: PE does q(p+1) while DVE/Act build Z(p)
            z0 = emit_q(0)
            z1 = emit_q(1)
            emit_v(0, z0)
            z2 = emit_q(2)
            emit_v(1, z1)
            z3 = emit_q(3)
            emit_v(2, z2)
            emit_v(3, z3)

    _legalize_waits(nc)
    return nc


def _programs(has_bias):
    key = ("ab", has_bias)
    if key not in _cache:
        _cache[key] = (_build_a(has_bias), _build_b(has_bias))
    return _cache[key]


def kernel(output, hidden_states, Wq, bq, Wk, bk, Wv, bv, gate, _trace=False):
    import ml_dtypes
    from concourse import bass_utils

    output = np.asarray(output, dtype=np.float32)
    hidden = np.asarray(hidden_states, dtype=np.float32)
    cst = _host_constants(
        np.asarray(Wq, np.float32), np.asarray(bq, np.float32),
        np.asarray(Wk, np.float32), np.asarray(bk, np.float32),
        np.asarray(Wv, np.float32), np.asarray(bv, np.float32),
        np.asarray(gate, np.float32))
    has_bias = cst["has_bias"]
    nca, ncb = _programs(has_bias)

    ac = np.ascontiguousarray
    chunks = [(c // 4, c % 4) for c in range(NCORES)]

    def ht_pack(b, j):
        hT = hidden[b, j * CHUNK:(j + 1) * CHUNK, :].T  # [1024, 2048]
        return ac(hT.reshape(8, 128, CHUNK).transpose(1, 0, 2)
                  .reshape(128, 8 * CHUNK)).astype(ml_dtypes.bfloat16)

    sharedA = {"MkP": cst["MkP"], "MvP": cst["MvP"]}
    if has_bias:
        sharedA["biask"] = cst["bk"]
        sharedA["biasv"] = cst["bv"]

    hts = [ht_pack(b, j) for (b, j) in chunks]
    in_a = []
    for c, (b, j) in enumerate(chunks):
        im = dict(sharedA)
        im["ht"] = hts[c]
        in_a.append(im)
    res_a = bass_utils.run_bass_kernel_spmd(
        nca, in_a, core_ids=list(range(NCORES)), trace=_trace)

    # host: causal prefix over per-chunk totals (fp32)
    cars = [np.asarray(res_a.results[c]["car"], np.float32)
            for c in range(NCORES)]
    inits = []
    for c, (b, j) in enumerate(chunks):
        p = np.zeros((128, 8), np.float32)
        for c2, (b2, j2) in enumerate(chunks):
            if b2 == b and j2 < j:
                p += cars[c2]
        inits.append(p)

    sharedB = {"MqP": cst["MqP"], "RP": cst["RP"]}
    if has_bias:
        sharedB["biasq"] = cst["bq"]

    in_b = []
    for c, (b, j) in enumerate(chunks):
        im = dict(sharedB)
        im["ht"] = hts[c]
        im["mem"] = res_a.results[c]["mem"]
        im["init"] = inits[c]
        im["outp"] = ac(output[b, j * CHUNK:(j + 1) * CHUNK, :])
        in_b.append(im)
    res_b = bass_utils.run_bass_kernel_spmd(
        ncb, in_b, core_ids=list(range(NCORES)), trace=_trace)

    out = np.empty((B, S, D), dtype=np.float32)
    for c, (b, j) in enumerate(chunks):
        out[b, j * CHUNK:(j + 1) * CHUNK, :] = res_b.results[c]["res"]
    if _trace:
        kernel._last = (res_a, res_b)
    return out
